# revision 15
# baseline (speedup 1.0000x reference)
"""Trainium2 Bass kernel for nn_BasicBlock_1709396984498.

Data-parallel over batch: 1 sample per NeuronCore (8 cores).
Per core: elementwise ADMM phase in fp32 planes [128,512], channel
gather/scatter via host-computed one-hot permutation, RDN conv stack
streamed through padded DRAM maps in bf16 (3x3 convs as 6 matmuls per
512-px tile: 3 row-shift-paired taps at K=128 + 3 singles at K=64).
"""

import json
import os
import sys

for _p in (
    "/root/.axon_site",
    "/root/.axon_site/_ro/trn_rl_repo",
    "/root/.axon_site/_ro/pypackages",
    "/opt/trn_rl_repo",
):
    if os.path.isdir(_p) and _p not in sys.path:
        sys.path.append(_p)

import ml_dtypes
import numpy as np

import concourse.bass as bass
import concourse.tile as tile
from concourse import mybir
from concourse.bass_utils import run_bass_kernel_spmd

F32 = mybir.dt.float32
BF16 = mybir.dt.bfloat16
ALU = mybir.AluOpType
ACT = mybir.ActivationFunctionType

GAMMA_1, GAMMA_2 = 0.3, 0.7
H = W = 256
HW = H * W
WP = W + 2  # padded width (258)
PADHW = (H + 2) * WP  # 66564
G = 64
R = 32  # strip rows
NSTRIP = H // R
SLEN = R * WP  # 8256 flat output positions per strip
NCORES = 8

_BF = ml_dtypes.bfloat16


def _split_sync_waits(nc, max_waits=1):
    """Walrus in this env rejects >1 sync-wait on CTRL ops; split overflow
    waits onto preceding same-engine NoOps (engines dispatch in order)."""
    import bass_rust

    m = json.loads(bass_rust.module_to_json_string(nc.m))
    n = [0]
    changed = False
    for f in m["functions"]:
        for bb in f["blocks"]:
            out = []
            for inst in bb["instructions"]:
                si = inst.get("sync_info")
                waits = (si or {}).get("on_wait") or []
                if len(waits) > max_waits:
                    changed = True
                    keep = waits[len(waits) - max_waits :]
                    over = waits[: len(waits) - max_waits]
                    for i in range(0, len(over), max_waits):
                        n[0] += 1
                        out.append(
                            {
                                "debug": inst.get("debug", 0),
                                "engine": inst["engine"],
                                "ins": [],
                                "outs": [],
                                "name": f"wsplit-{n[0]}",
                                "opcode": "NoOp",
                                "sync_info": {
                                    "on_update": [],
                                    "on_wait": over[i : i + max_waits],
                                },
                            }
                        )
                    si["on_wait"] = keep
                out.append(inst)
            bb["instructions"] = out
    if changed:
        nc.m = bass_rust.module_from_json_string(json.dumps(m))
    return nc


def _build_program(sc):
    """sc: dict of python-float scalars (beta, eta, lam1, lam2, wt0..2)."""
    beta, eta = sc["beta"], sc["eta"]
    lam1, lam2 = sc["lam1"], sc["lam2"]
    wt0, wt1, wt2 = sc["wt0"], sc["wt1"], sc["wt2"]

    nc = bass.Bass("TRN2", target_bir_lowering=False, debug=False,
                   num_devices=NCORES)

    def din(name, shape, dt=F32):
        return nc.dram_tensor(name, shape, dt, kind="ExternalInput")

    def dout(name, shape, dt=F32):
        return nc.dram_tensor(name, shape, dt, kind="ExternalOutput")

    I3 = din("I3", [3, 128, 512])
    tp3 = din("tp3", [3, 128, 512])
    bp3 = din("bp3", [3, 128, 512])
    t3 = din("t3", [3, 128, 512])
    J3 = din("J3", [3, 128, 512])
    Y3 = din("Y3", [3, 128, 512])
    Z3 = din("Z3", [3, 128, 512])
    Q3 = din("Q3", [3, 128, 512])
    R3 = din("R3", [3, 128, 512])
    u1 = din("u1", [128, 512])
    v1 = din("v1", [128, 512])
    w11 = din("w11", [128, 512])
    w21 = din("w21", [128, 512])
    psc = din("psc", [16])  # P flat (9), jlbar (1), pad

    wq = {}
    for nm in ("sfe2", "d0", "d1", "g2"):
        wq[nm + "_p"] = din("w_" + nm + "_p", [128, 192], BF16)
        wq[nm + "_s"] = din("w_" + nm + "_s", [64, 192], BF16)
    wq["out_p"] = din("w_out_p", [128, 9], BF16)
    wq["out_s"] = din("w_out_s", [64, 9], BF16)
    wq["sfe1"] = din("w_sfe1", [27, 64], BF16)
    for nm in ("l0", "l1", "g1"):
        wq[nm] = din("w_" + nm, [128, 64], BF16)
    bq = {}
    for nm in ("sfe1", "sfe2", "d0", "l0", "d1", "l1", "g1", "g2"):
        bq[nm] = din("b_" + nm, [64])
    bq["out"] = din("b_out", [3])

    o_j = dout("o_j", [3, 128, 512])
    o_q = dout("o_q", [3, 128, 512])
    o_z = dout("o_z", [3, 128, 512])
    o_r = dout("o_r", [3, 128, 512])
    o_t1 = dout("o_t1", [128, 512])
    o_u = dout("o_u", [128, 512])
    o_v = dout("o_v", [128, 512])
    o_w1 = dout("o_w1", [128, 512])
    o_w2 = dout("o_w2", [128, 512])
    o_bm = dout("o_bm", [1, 3])

    with tile.TileContext(nc) as tc:
        from contextlib import ExitStack

        with ExitStack() as ctx:
            persist = ctx.enter_context(tc.tile_pool(name="persist", bufs=1))
            dmaps = ctx.enter_context(
                tc.tile_pool(name="dmaps", bufs=1, space="DRAM"))
            psA = ctx.enter_context(
                tc.tile_pool(name="psA", bufs=5, space="PSUM"))
            psB = ctx.enter_context(
                tc.tile_pool(name="psB", bufs=1, space="PSUM"))

            v = nc.vector
            s = nc.scalar
            dma = nc.sync.dma_start

            # ---- persistent tiles ----
            wt = {}
            for nm, h in wq.items():
                wt[nm] = persist.tile(list(h.shape), BF16, tag="w_" + nm, name="w_" + nm)
                dma(wt[nm][:], h[:])
            bt = {}
            for nm, h in bq.items():
                n = h.shape[0]
                bt[nm] = persist.tile([n, 1], F32, tag="b_" + nm, name="b_" + nm)
                dma(bt[nm][:], h[:].rearrange("(p c) -> p c", c=1))

            ones_row = persist.tile([1, 128], F32, tag="ones_row", name="ones_row")
            v.memset(ones_row[:], 1.0)
            ones_col = persist.tile([128, 1], F32, tag="ones_col", name="ones_col")
            v.memset(ones_col[:], 1.0)
            zt = persist.tile([64, 264], BF16, tag="zt", name="zt")
            v.memset(zt[:], 0.0)

            pj_src = persist.tile([1, 16], F32, tag="pj_src", name="pj_src")
            dma(pj_src[0:1, 0:16], psc[:].rearrange("(p c) -> p c", p=1))
            ps_p = psB.tile([128, 16], F32, tag="t_bc", name="t_bc")
            nc.tensor.matmul(ps_p[:], ones_row[:], pj_src[:],
                             start=True, stop=True)
            pscal = persist.tile([128, 16], F32, tag="pscal", name="pscal")
            s.copy(pscal[:], ps_p[:])

            t1 = persist.tile([128, 512], F32, tag="t1", name="t1")
            bscal = persist.tile([128, 4], F32, tag="bscal", name="bscal")
            msrc = persist.tile([1, 4], F32, tag="msrc", name="msrc")
            v.memset(msrc[:], 0.0)

            # ---- DRAM feature maps (bf16, padded 258x258) ----
            maps = {}
            maps["zin"] = dmaps.tile([3, PADHW], BF16, tag="zin", name="zin")
            for nm in ("sfe1", "h0", "d0", "h1", "d1", "h2", "g", "g2"):
                maps[nm] = dmaps.tile([64, PADHW], BF16, tag=nm, name=nm)
            zmap = dmaps.tile([3, HW], BF16, tag="zmap", name="zmap")

            # zero the borders of maps read by 3x3 convs
            for nm in ("zin", "sfe1", "h0", "h1", "g", "g2"):
                mp = maps[nm]
                C = mp.shape[0]
                m3 = mp[:].rearrange("p (r c) -> p r c", c=WP)
                dma(mp[:, 0:WP], zt[0:C, 0:WP])
                dma(mp[:, (WP - 1) * WP : WP * WP], zt[0:C, 0:WP])
                dma(m3[:, :, 0:1],
                    zt[0:C, 0:WP].rearrange("p (r c) -> p r c", c=1))
                dma(m3[:, :, WP - 1 : WP],
                    zt[0:C, 0:WP].rearrange("p (r c) -> p r c", c=1))

            def pA(k):
                return pscal[:, k : k + 1]

            # ================= E1 + E2 (elementwise) =================
            with tc.tile_pool(name="ew", bufs=1) as ew:
                def pl(name):
                    return ew.tile([128, 512], F32, tag="pl_" + name, name="pl_" + name)

                pj = [pl(f"J{c}") for c in range(3)]
                for c in range(3):
                    dma(pj[c][:], J3[c])
                pu, pv = pl("u"), pl("v")
                pw1, pw2 = pl("w1"), pl("w2")
                dma(pu[:], u1[:]); dma(pv[:], v1[:])
                dma(pw1[:], w11[:]); dma(pw2[:], w21[:])

                js = [pl(f"js{k}") for k in range(3)]
                for k in range(3):
                    v.tensor_scalar_mul(js[k][:], pj[0][:], pA(3 * k + 0))
                    v.scalar_tensor_tensor(js[k][:], pj[1][:], pA(3 * k + 1),
                                           js[k][:], ALU.mult, ALU.add)
                    v.scalar_tensor_tensor(js[k][:], pj[2][:], pA(3 * k + 2),
                                           js[k][:], ALU.mult, ALU.add)
                jl = js[2]

                x1, x2 = pl("x1"), pl("x2")
                v.scalar_tensor_tensor(x1[:], jl[:], -1.0, pu[:],
                                       ALU.mult, ALU.add)
                v.tensor_scalar_add(x1[:], x1[:], pA(9))
                v.scalar_tensor_tensor(x2[:], jl[:], -1.0, pv[:],
                                       ALU.mult, ALU.add)
                v.tensor_scalar_add(x2[:], x2[:], pA(9))

                tmp, tmp2 = pl("tmp"), pl("tmp2")
                # u_new / v_new (soft threshold)
                v.tensor_scalar(tmp[:], x1[:], 1.0 / lam1, -1.0 / lam1,
                                ALU.min, ALU.max)
                v.tensor_sub(tmp[:], x1[:], tmp[:])
                dma(o_u[:], tmp[:])
                v.tensor_scalar(tmp[:], x2[:], 1.0 / lam2, -1.0 / lam2,
                                ALU.min, ALU.max)
                v.tensor_sub(tmp[:], x2[:], tmp[:])
                dma(o_v[:], tmp[:])
                # w_new (clip)
                v.tensor_scalar(tmp[:], x1[:], lam1, 1.0, ALU.mult, ALU.min)
                v.tensor_scalar_max(tmp[:], tmp[:], -1.0)
                dma(o_w1[:], tmp[:])
                v.tensor_scalar(tmp[:], x2[:], lam2, 1.0, ALU.mult, ALU.min)
                v.tensor_scalar_max(tmp[:], tmp[:], -1.0)
                dma(o_w2[:], tmp[:])

                # new sorted-channel values
                jm_n, js_n = pl("jm_n"), pl("js_n")
                v.scalar_tensor_tensor(tmp[:], pw1[:], -1.0 / lam1, x1[:],
                                       ALU.mult, ALU.add)
                v.tensor_mul(tmp[:], tmp[:], jl[:])
                v.tensor_add(jm_n[:], tmp[:], js[1][:])
                v.scalar_tensor_tensor(tmp[:], pw2[:], -1.0 / lam2, x2[:],
                                       ALU.mult, ALU.add)
                v.tensor_mul(tmp[:], tmp[:], jl[:])
                v.tensor_add(js_n[:], tmp[:], js[0][:])

                comp = [js_n, jm_n, jl]
                jsc = [pl(f"jsc{c}") for c in range(3)]
                for c in range(3):
                    v.tensor_scalar_mul(jsc[c][:], comp[0][:], pA(0 + c))
                    v.scalar_tensor_tensor(jsc[c][:], comp[1][:], pA(3 + c),
                                           jsc[c][:], ALU.mult, ALU.add)
                    v.scalar_tensor_tensor(jsc[c][:], comp[2][:], pA(6 + c),
                                           jsc[c][:], ALU.mult, ALU.add)

                # ---- E2 ----
                pi = [pl(f"I{c}") for c in range(3)]
                pt = [pl(f"t{c}") for c in range(3)]
                for c in range(3):
                    dma(pi[c][:], I3[c]); dma(pt[c][:], t3[c])
                pbp = pl("bp")
                prt = pl("prt")
                for c in range(3):
                    dma(pbp[:], bp3[c])
                    omt = pl("omt")
                    v.tensor_scalar(omt[:], pt[c][:], -1.0, 1.0,
                                    ALU.mult, ALU.add)
                    v.tensor_mul(tmp[:], jsc[c][:], pt[c][:])
                    v.tensor_sub(tmp[:], tmp[:], pi[c][:])
                    v.tensor_mul(tmp[:], tmp[:], omt[:])
                    v.scalar_tensor_tensor(tmp[:], pbp[:], GAMMA_1, tmp[:],
                                           ALU.mult, ALU.subtract)
                    v.tensor_mul(tmp2[:], omt[:], omt[:])
                    v.tensor_scalar_add(tmp2[:], tmp2[:], GAMMA_1)
                    rcp = pl("rcp")
                    v.reciprocal(rcp[:], tmp2[:])
                    v.tensor_mul(tmp[:], tmp[:], rcp[:])
                    v.tensor_reduce(prt[:, 0:1], tmp[:],
                                    mybir.AxisListType.X, ALU.add)
                    ps_m = psB.tile([1, 1], F32, tag="t_m", name="t_m")
                    nc.tensor.matmul(ps_m[:], prt[:, 0:1], ones_col[:],
                                     start=True, stop=True)
                    s.mul(msrc[0:1, c : c + 1], ps_m[:], 1.0 / HW)
                ps_b = psB.tile([128, 4], F32, tag="t_bc2", name="t_bc2")
                nc.tensor.matmul(ps_b[:], ones_row[:], msrc[:],
                                 start=True, stop=True)
                s.copy(bscal[:], ps_b[:])
                dma(o_bm[:], msrc[0:1, 0:3])

                pzc, prc = pl("Zc"), pl("Rc")
                tt = [pl(f"tt{c}") for c in range(3)]
                for c in range(3):
                    dma(pzc[:], Z3[c]); dma(prc[:], R3[c])
                    jb = pl("jb")
                    v.tensor_scalar_sub(jb[:], jsc[c][:],
                                        bscal[:, c : c + 1])
                    v.tensor_scalar(tmp[:], pi[c][:], -1.0,
                                    bscal[:, c : c + 1], ALU.mult, ALU.add)
                    v.tensor_mul(tmp[:], tmp[:], jb[:])
                    v.scalar_tensor_tensor(tmp2[:], pzc[:], eta, prc[:],
                                           ALU.mult, ALU.subtract)
                    ptp = pl("tpc")
                    dma(ptp[:], tp3[c])
                    v.scalar_tensor_tensor(tmp2[:], ptp[:], GAMMA_2, tmp2[:],
                                           ALU.mult, ALU.add)
                    v.tensor_sub(tmp2[:], tmp2[:], tmp[:])
                    v.tensor_mul(tmp[:], jb[:], jb[:])
                    v.tensor_scalar_add(tmp[:], tmp[:], GAMMA_2 + eta)
                    rcp2 = pl("rcp")
                    v.reciprocal(rcp2[:], tmp[:])
                    v.tensor_mul(tt[c][:], tmp2[:], rcp2[:])

                v.tensor_scalar_mul(tmp[:], tt[0][:], wt0)
                v.scalar_tensor_tensor(tmp[:], tt[1][:], wt1, tmp[:],
                                       ALU.mult, ALU.add)
                v.scalar_tensor_tensor(tmp[:], tt[2][:], wt2, tmp[:],
                                       ALU.mult, ALU.add)
                s.activation(t1[:], tmp[:], ACT.Relu)
                dma(o_t1[:], t1[:])

                rdj = pl("rdj")
                v.tensor_mul(tmp[:], t1[:], t1[:])
                v.tensor_scalar_add(tmp[:], tmp[:], beta)
                v.reciprocal(rdj[:], tmp[:])
                omt1 = pl("omt1")
                v.tensor_scalar(omt1[:], t1[:], -1.0, 1.0, ALU.mult, ALU.add)

                py, pq = pl("Yc"), pl("Qc")
                for c in range(3):
                    dma(py[:], Y3[c]); dma(pq[:], Q3[c])
                    v.tensor_scalar_mul(tmp[:], omt1[:], bscal[:, c : c + 1])
                    v.tensor_sub(tmp[:], tmp[:], pi[c][:])
                    v.tensor_mul(tmp[:], tmp[:], t1[:])
                    v.scalar_tensor_tensor(tmp2[:], py[:], beta, pq[:],
                                           ALU.mult, ALU.subtract)
                    v.tensor_sub(tmp2[:], tmp2[:], tmp[:])
                    jn = pl("jn")
                    v.tensor_mul(jn[:], tmp2[:], rdj[:])
                    dma(o_j[c], jn[:])
                    v.tensor_sub(tmp[:], jn[:], py[:])
                    v.scalar_tensor_tensor(tmp[:], tmp[:], beta, pq[:],
                                           ALU.mult, ALU.add)
                    dma(o_q[c], tmp[:])
                    # Zin
                    dma(prc[:], R3[c])
                    v.scalar_tensor_tensor(tmp[:], prc[:], 1.0 / eta, t1[:],
                                           ALU.mult, ALU.add)
                    zb = ew.tile([128, 512], BF16, tag="pl_zinbf", name="pl_zinbf")
                    zb_b = zb[:]
                    v.tensor_copy(zb_b, tmp[:])
                    dst = maps["zin"][c : c + 1, :].rearrange(
                        "a (r c) -> a r c", c=WP)[0:1, 1:257, 1:257]
                    dst = dst.rearrange("a (p r) c -> (a p) r c", p=128)
                    src = zb_b.rearrange("p (r c) -> p r c", c=256)
                    dma(dst, src)

            # ================= RDN conv stack =================
            CIN_COLS = (R + 2) * WP + 12  # 8784
            with tc.tile_pool(name="cin", bufs=2) as cin, \
                 tc.tile_pool(name="cout", bufs=2) as cout, \
                 tc.tile_pool(name="cres", bufs=2) as cres:

                def tiles_of_strip():
                    q0 = 0
                    out = []
                    while q0 < SLEN:
                        out.append((q0, min(512, SLEN - q0)))
                        q0 += 512
                    return out

                def store_interior(src_buf, dst_map, y0, Cn):
                    src = src_buf[:].rearrange("p (r c) -> p r c", c=WP)
                    src = src[:, :, 1:257]
                    dst = dst_map[:].rearrange("p (r c) -> p r c", c=WP)
                    dst = dst[:, y0 + 1 : y0 + 1 + R, 1:257]
                    dma(dst, src)

                def conv3x3(src_map, dst_map, wp_t, ws_t, bias_t, relu,
                            resid_map=None):
                    M = 64
                    for st in range(NSTRIP):
                        y0 = st * R
                        it = cin.tile([128, CIN_COLS], BF16, tag="cin", name="cin")
                        dma(it[0:64, 1 : 1 + (R + 2) * WP],
                            src_map[:, y0 * WP : (y0 + R + 2) * WP])
                        dma(it[64:128, 1 : 1 + (R + 1) * WP],
                            it[0:64, 1 + WP : 1 + (R + 2) * WP])
                        ob = cout.tile([64, SLEN], BF16, tag="cout", name="cout")
                        if resid_map is not None:
                            rs = cres.tile([64, SLEN], BF16, tag="cres", name="cres")
                            dma(rs[:], resid_map[:,
                                (y0 + 1) * WP : (y0 + 1) * WP + SLEN])
                        for (q0, n) in tiles_of_strip():
                            ps = psA.tile([64, n], F32, tag="ps", name="ps")
                            for j in range(3):
                                nc.tensor.matmul(
                                    ps[:], wp_t[:, j * M : (j + 1) * M],
                                    it[0:128, q0 + j : q0 + j + n],
                                    start=(j == 0), stop=False)
                            for j in range(3):
                                nc.tensor.matmul(
                                    ps[:], ws_t[0:64, j * M : (j + 1) * M],
                                    it[0:64, q0 + 516 + j : q0 + 516 + j + n],
                                    start=False, stop=(j == 2))
                            obs = ob[:, q0 : q0 + n]
                            s.activation(obs, ps[:],
                                         ACT.Relu if relu else ACT.Identity,
                                         bias=bias_t[:])
                            if resid_map is not None:
                                v.tensor_add(obs, obs, rs[:, q0 : q0 + n])
                        store_interior(ob, dst_map, y0, 64)

                def conv1x1(a_map, b_map, dst_map, w_t, bias_t, resid):
                    for st in range(NSTRIP):
                        y0 = st * R
                        it = cin.tile([128, CIN_COLS], BF16, tag="cin", name="cin")
                        dma(it[0:64, 0:SLEN],
                            a_map[:, (y0 + 1) * WP : (y0 + 1) * WP + SLEN])
                        dma(it[64:128, 0:SLEN],
                            b_map[:, (y0 + 1) * WP : (y0 + 1) * WP + SLEN])
                        ob = cout.tile([64, SLEN], BF16, tag="cout", name="cout")
                        for (q0, n) in tiles_of_strip():
                            ps = psA.tile([64, n], F32, tag="ps", name="ps")
                            nc.tensor.matmul(ps[:], w_t[:],
                                             it[0:128, q0 : q0 + n],
                                             start=True, stop=True)
                            obs = ob[:, q0 : q0 + n]
                            s.activation(obs, ps[:], ACT.Identity,
                                         bias=bias_t[:])
                            if resid:
                                v.tensor_add(obs, obs, it[0:64, q0 : q0 + n])
                        store_interior(ob, dst_map, y0, 64)

                def conv_sfe1(src_map, dst_map, w_t, bias_t):
                    D = 2
                    for st in range(NSTRIP):
                        y0 = st * R
                        it = cin.tile([27, CIN_COLS], BF16, tag="cin9", name="cin9")
                        for a in range(3):
                            for b in range(3):
                                tk = a * 3 + b
                                src0 = y0 * WP + a * WP + b - 1
                                L = min(SLEN + 8, PADHW - max(src0, 0))
                                d0_ = D + (max(src0, 0) - src0)
                                dma(it[3 * tk : 3 * tk + 3, d0_ : d0_ + L],
                                    src_map[:, max(src0, 0) : max(src0, 0) + L])
                        ob = cout.tile([64, SLEN], BF16, tag="cout", name="cout")
                        for (q0, n) in tiles_of_strip():
                            ps = psA.tile([64, n], F32, tag="ps", name="ps")
                            nc.tensor.matmul(ps[:], w_t[:],
                                             it[0:27, D + q0 : D + q0 + n],
                                             start=True, stop=True)
                            s.activation(ob[:, q0 : q0 + n], ps[:],
                                         ACT.Identity, bias=bias_t[:])
                        store_interior(ob, dst_map, y0, 64)

                def conv_out(src_map, wp_t, ws_t, bias_t):
                    for st in range(NSTRIP):
                        y0 = st * R
                        it = cin.tile([128, CIN_COLS], BF16, tag="cin", name="cin")
                        dma(it[0:64, 1 : 1 + (R + 2) * WP],
                            src_map[:, y0 * WP : (y0 + R + 2) * WP])
                        dma(it[64:128, 1 : 1 + (R + 1) * WP],
                            it[0:64, 1 + WP : 1 + (R + 2) * WP])
                        zb = cres.tile([3, SLEN], BF16, tag="zbuf", name="zbuf", bufs=1)
                        for (q0, n) in tiles_of_strip():
                            ps = psA.tile([3, n], F32, tag="ps", name="ps")
                            for j in range(3):
                                nc.tensor.matmul(
                                    ps[:], wp_t[:, j * 3 : (j + 1) * 3],
                                    it[0:128, q0 + j : q0 + j + n],
                                    start=(j == 0), stop=False)
                            for j in range(3):
                                nc.tensor.matmul(
                                    ps[:], ws_t[0:64, j * 3 : (j + 1) * 3],
                                    it[0:64, q0 + 516 + j : q0 + 516 + j + n],
                                    start=False, stop=(j == 2))
                            s.activation(zb[:, q0 : q0 + n], ps[:],
                                         ACT.Identity, bias=bias_t[:])
                        src = zb[:].rearrange("p (r c) -> p r c", c=WP)
                        src = src[:, :, 1:257]
                        dst = zmap[:].rearrange("p (r c) -> p r c", c=256)
                        dst = dst[:, y0 : y0 + R, :]
                        dma(dst, src)

                conv_sfe1(maps["zin"], maps["sfe1"], wt["sfe1"], bt["sfe1"])
                conv3x3(maps["sfe1"], maps["h0"], wt["sfe2_p"], wt["sfe2_s"],
                        bt["sfe2"], relu=False)
                conv3x3(maps["h0"], maps["d0"], wt["d0_p"], wt["d0_s"],
                        bt["d0"], relu=True)
                conv1x1(maps["h0"], maps["d0"], maps["h1"], wt["l0"],
                        bt["l0"], resid=True)
                conv3x3(maps["h1"], maps["d1"], wt["d1_p"], wt["d1_s"],
                        bt["d1"], relu=True)
                conv1x1(maps["h1"], maps["d1"], maps["h2"], wt["l1"],
                        bt["l1"], resid=True)
                conv1x1(maps["h1"], maps["h2"], maps["g"], wt["g1"],
                        bt["g1"], resid=False)
                conv3x3(maps["g"], maps["g2"], wt["g2_p"], wt["g2_s"],
                        bt["g2"], relu=False, resid_map=maps["sfe1"])
                conv_out(maps["g2"], wt["out_p"], wt["out_s"], bt["out"])

            # ================= E3 =================
            with tc.tile_pool(name="e3", bufs=2) as e3:
                for c in range(3):
                    rp = e3.tile([128, 512], F32, tag="rp", name="rp")
                    dma(rp[:], R3[c])
                    zpb = e3.tile([128, 512], BF16, tag="zpb", name="zpb")
                    dma(zpb[:], zmap[:].rearrange(
                        "a (p c) -> a p c", c=512)[c])
                    zp = e3.tile([128, 512], F32, tag="zp", name="zp")
                    v.tensor_copy(zp[:], zpb[:])
                    dma(o_z[c], zp[:])
                    rn = e3.tile([128, 512], F32, tag="rn", name="rn")
                    v.tensor_sub(rn[:], t1[:], zp[:])
                    v.scalar_tensor_tensor(rn[:], rn[:], eta, rp[:],
                                           ALU.mult, ALU.add)
                    dma(o_r[c], rn[:])

    nc.finalize()
    _split_sync_waits(nc, max_waits=1)
    return nc


_CACHE = {}


def _pack_weights(params):
    def bf(x):
        return np.ascontiguousarray(x.astype(_BF))

    out = {}
    for nm, key in (("sfe2", "sfe2_w"), ("d0", "rdb0_dense_w"),
                    ("d1", "rdb1_dense_w"), ("g2", "gff2_w")):
        Wt = np.asarray(params[key], np.float32)  # [64,64,3,3]
        wp = np.zeros((128, 192), np.float32)
        ws = np.zeros((64, 192), np.float32)
        for j in range(3):
            wp[0:64, j * 64 : (j + 1) * 64] = Wt[:, :, 0, j].T
            wp[64:128, j * 64 : (j + 1) * 64] = Wt[:, :, 1, j].T
            ws[0:64, j * 64 : (j + 1) * 64] = Wt[:, :, 2, j].T
        out["w_" + nm + "_p"] = bf(wp)
        out["w_" + nm + "_s"] = bf(ws)
    Wt = np.asarray(params["out_w"], np.float32)  # [3,64,3,3]
    wp = np.zeros((128, 9), np.float32)
    ws = np.zeros((64, 9), np.float32)
    for j in range(3):
        wp[0:64, j * 3 : (j + 1) * 3] = Wt[:, :, 0, j].T
        wp[64:128, j * 3 : (j + 1) * 3] = Wt[:, :, 1, j].T
        ws[0:64, j * 3 : (j + 1) * 3] = Wt[:, :, 2, j].T
    out["w_out_p"] = bf(wp)
    out["w_out_s"] = bf(ws)
    Wt = np.asarray(params["sfe1_w"], np.float32)  # [64,3,3,3]
    w9 = np.zeros((27, 64), np.float32)
    for a in range(3):
        for b in range(3):
            tk = a * 3 + b
            w9[3 * tk : 3 * tk + 3, :] = Wt[:, :, a, b].T
    out["w_sfe1"] = bf(w9)
    for nm, key in (("l0", "rdb0_lff_w"), ("l1", "rdb1_lff_w"),
                    ("g1", "gff1_w")):
        Wt = np.asarray(params[key], np.float32)  # [64,128,1,1]
        out["w_" + nm] = bf(Wt[:, :, 0, 0].T)
    for nm, key in (("sfe1", "sfe1_b"), ("sfe2", "sfe2_b"),
                    ("d0", "rdb0_dense_b"), ("l0", "rdb0_lff_b"),
                    ("d1", "rdb1_dense_b"), ("l1", "rdb1_lff_b"),
                    ("g1", "gff1_b"), ("g2", "gff2_b"), ("out", "out_b")):
        out["b_" + nm] = np.ascontiguousarray(
            np.asarray(params[key], np.float32))
    return out


def kernel(**inputs):
    params = inputs["params"]
    beta = float(np.asarray(params["beta"])[0])
    eta = float(np.asarray(params["eta"])[0])
    lam1 = float(np.asarray(params["lambda_1"])[0])
    lam2 = float(np.asarray(params["lambda_2"])[0])
    wtd = np.asarray(params["t1d_w"], np.float32)[0, :, 0, 0]
    sc = dict(beta=beta, eta=eta, lam1=lam1, lam2=lam2,
              wt0=float(wtd[0]), wt1=float(wtd[1]), wt2=float(wtd[2]))
    key = tuple(sorted(sc.items()))
    if key not in _CACHE:
        _CACHE[key] = _build_program(sc)
    nc = _CACHE[key]

    wpk = _pack_weights(params)

    f32 = lambda x: np.ascontiguousarray(np.asarray(x, np.float32))
    J = f32(inputs["J"])
    means = J.mean(axis=(2, 3), dtype=np.float32)  # [8,3]
    idx = np.argsort(means, axis=1, kind="stable")

    in_maps = []
    for b in range(NCORES):
        P = np.zeros((3, 3), np.float32)
        for k in range(3):
            P[k, idx[b, k]] = 1.0
        psc = np.zeros(16, np.float32)
        psc[0:9] = P.reshape(-1)
        psc[9] = means[b, idx[b, 2]]
        m = {
            "I3": f32(inputs["I"][b]).reshape(3, 128, 512),
            "tp3": f32(inputs["t_p"][b]).reshape(3, 128, 512),
            "bp3": f32(inputs["B_p"][b]).reshape(3, 128, 512),
            "t3": f32(inputs["t"][b]).reshape(3, 128, 512),
            "J3": J[b].reshape(3, 128, 512),
            "Y3": f32(inputs["Y"][b]).reshape(3, 128, 512),
            "Z3": f32(inputs["Z"][b]).reshape(3, 128, 512),
            "Q3": f32(inputs["Q"][b]).reshape(3, 128, 512),
            "R3": f32(inputs["R"][b]).reshape(3, 128, 512),
            "u1": f32(inputs["u"][b]).reshape(128, 512),
            "v1": f32(inputs["v"][b]).reshape(128, 512),
            "w11": f32(inputs["w_1"][b]).reshape(128, 512),
            "w21": f32(inputs["w_2"][b]).reshape(128, 512),
            "psc": psc,
        }
        m.update(wpk)
        in_maps.append(m)

    trace = bool(int(os.environ.get("KERNEL_TRACE", "0")))
    tkw = {}
    if trace:
        try:
            sys.path.insert(0, "/root/problem/work")
            import profhook

            profhook.install()
            tkw = dict(trace=True, tmpdir=os.environ.get(
                "KERNEL_TRACE_DIR", "/root/problem/work/trace_out"))
        except Exception:
            tkw = {}
    r = run_bass_kernel_spmd(nc, in_maps, list(range(NCORES)), **tkw)
    res = r.results
    if trace and getattr(r, "exec_time_ns", None) is not None:
        kernel.last_exec_time_ns = r.exec_time_ns

    B = np.empty((8, 3, H, W), np.float32)
    t_new = np.empty((8, 3, H, W), np.float32)
    Jn = np.empty((8, 3, H, W), np.float32)
    Zn = np.empty((8, 3, H, W), np.float32)
    Qn = np.empty((8, 3, H, W), np.float32)
    Rn = np.empty((8, 3, H, W), np.float32)
    un = np.empty((8, 1, H, W), np.float32)
    vn = np.empty((8, 1, H, W), np.float32)
    w1n = np.empty((8, 1, H, W), np.float32)
    w2n = np.empty((8, 1, H, W), np.float32)
    for b in range(NCORES):
        o = res[b]
        B[b] = o["o_bm"].reshape(3, 1, 1)
        t_new[b] = o["o_t1"].reshape(1, H, W)
        Jn[b] = o["o_j"].reshape(3, H, W)
        Zn[b] = o["o_z"].reshape(3, H, W)
        Qn[b] = o["o_q"].reshape(3, H, W)
        Rn[b] = o["o_r"].reshape(3, H, W)
        un[b, 0] = o["o_u"].reshape(H, W)
        vn[b, 0] = o["o_v"].reshape(H, W)
        w1n[b, 0] = o["o_w1"].reshape(H, W)
        w2n[b, 0] = o["o_w2"].reshape(H, W)
    Y = f32(inputs["Y"])
    return (B, t_new, Jn, Y, Zn, Qn, Rn, un, vn, w1n, w2n,
            f32(params["beta"]))


# revision 24
# speedup vs baseline: 1.3241x; 1.3241x over previous
"""Trainium2 Bass kernel for nn_BasicBlock_1709396984498.

Data-parallel over batch: 1 sample per NeuronCore (8 cores).
Per core: elementwise ADMM phase in fp32 planes [128,512], channel
gather/scatter via host-computed one-hot permutation, RDN conv stack
streamed through padded DRAM maps in bf16 (3x3 convs as 6 matmuls per
512-px tile: 3 row-shift-paired taps at K=128 + 3 singles at K=64).
"""

import json
import os
import sys

for _p in (
    "/root/.axon_site",
    "/root/.axon_site/_ro/trn_rl_repo",
    "/root/.axon_site/_ro/pypackages",
    "/opt/trn_rl_repo",
):
    if os.path.isdir(_p) and _p not in sys.path:
        sys.path.append(_p)

import ml_dtypes
import numpy as np

import concourse.bass as bass
import concourse.bass_isa as bass_isa
import concourse.tile as tile
from concourse import mybir
import concourse.bass_utils as _bu
from concourse.bass_utils import run_bass_kernel_spmd


F32 = mybir.dt.float32
BF16 = mybir.dt.bfloat16
ALU = mybir.AluOpType
ACT = mybir.ActivationFunctionType

GAMMA_1, GAMMA_2 = 0.3, 0.7
H = W = 256
HW = H * W
WP = W + 2  # padded width (258)
PADHW = (H + 2) * WP  # 66564
G = 64
R = 32  # strip rows
NSTRIP = H // R
SLEN = R * WP  # 8256 flat output positions per strip
NCORES = 8

_BF = ml_dtypes.bfloat16


def _split_sync_waits(nc, max_waits=1):
    """Post-pass on the final BIR:
    1) Replace InstLdweights whose weights operand matches the previous
       PE weight load (with only Matmults between) by a NoOp that keeps
       the same sync_info — weight-stationary reuse the toolchain's
       disabled ldw-opt would otherwise do.
    2) Walrus in this env rejects >1 sync-wait on CTRL ops; split
       overflow waits onto preceding same-engine NoOps."""
    import bass_rust

    m = json.loads(bass_rust.module_to_json_string(nc.m))
    for f in m["functions"]:
        for bb in f["blocks"]:
            last_w = None
            for inst in bb["instructions"]:
                op = inst["opcode"]
                if inst.get("engine") != "PE":
                    continue
                if op == "Ldweights":
                    key = json.dumps(inst.get("ins"), sort_keys=True)
                    if key == last_w and not (
                        (inst.get("sync_info") or {}).get("on_update")
                    ):
                        inst["opcode"] = "NoOp"
                        inst["ins"] = []
                        inst["outs"] = []
                    else:
                        last_w = key
                elif op not in ("Matmult",):
                    last_w = None
    n = [0]
    changed = True
    for f in m["functions"]:
        for bb in f["blocks"]:
            out = []
            for inst in bb["instructions"]:
                si = inst.get("sync_info")
                waits = (si or {}).get("on_wait") or []
                if len(waits) > max_waits:
                    changed = True
                    keep = waits[len(waits) - max_waits :]
                    over = waits[: len(waits) - max_waits]
                    for i in range(0, len(over), max_waits):
                        n[0] += 1
                        out.append(
                            {
                                "debug": inst.get("debug", 0),
                                "engine": inst["engine"],
                                "ins": [],
                                "outs": [],
                                "name": f"wsplit-{n[0]}",
                                "opcode": "NoOp",
                                "sync_info": {
                                    "on_update": [],
                                    "on_wait": over[i : i + max_waits],
                                },
                            }
                        )
                    si["on_wait"] = keep
                out.append(inst)
            bb["instructions"] = out
    if changed:
        nc.m = bass_rust.module_from_json_string(json.dumps(m))
    return nc


def _build_program(sc):
    """sc: dict of python-float scalars (beta, eta, lam1, lam2, wt0..2)."""
    beta, eta = sc["beta"], sc["eta"]
    lam1, lam2 = sc["lam1"], sc["lam2"]
    wt0, wt1, wt2 = sc["wt0"], sc["wt1"], sc["wt2"]

    nc = bass.Bass("TRN2", target_bir_lowering=False, debug=False,
                   num_devices=NCORES)

    def din(name, shape, dt=F32):
        return nc.dram_tensor(name, shape, dt, kind="ExternalInput")

    def dout(name, shape, dt=F32):
        return nc.dram_tensor(name, shape, dt, kind="ExternalOutput")

    I3 = din("I3", [3, 128, 512])
    tp3 = din("tp3", [3, 128, 512])
    bp3 = din("bp3", [3, 128, 512])
    t3 = din("t3", [3, 128, 512])
    J3 = din("J3", [3, 128, 512])
    Y3 = din("Y3", [3, 128, 512])
    Z3 = din("Z3", [3, 128, 512])
    Q3 = din("Q3", [3, 128, 512])
    R3 = din("R3", [3, 128, 512])
    u1 = din("u1", [128, 512])
    v1 = din("v1", [128, 512])
    w11 = din("w11", [128, 512])
    w21 = din("w21", [128, 512])
    psc = din("psc", [16])  # P flat (9), jlbar (1), pad

    wq = {}
    for nm in ("sfe2", "d0", "d1", "g2"):
        wq[nm + "_p"] = din("w_" + nm + "_p", [128, 192], BF16)
        wq[nm + "_s"] = din("w_" + nm + "_s", [64, 192], BF16)
    wq["out_p"] = din("w_out_p", [128, 9], BF16)
    wq["out_s"] = din("w_out_s", [64, 9], BF16)
    wq["sfe1"] = din("w_sfe1", [27, 64], BF16)
    for nm in ("l0", "l1", "g1"):
        wq[nm] = din("w_" + nm, [128, 64], BF16)
    bq = {}
    for nm in ("sfe1", "sfe2", "d0", "l0", "d1", "l1", "g1", "g2"):
        bq[nm] = din("b_" + nm, [64])
    bq["out"] = din("b_out", [3])

    o_j = dout("o_j", [3, 128, 512])
    o_q = dout("o_q", [3, 128, 512])
    o_z = dout("o_z", [3, 128, 512])
    o_r = dout("o_r", [3, 128, 512])
    o_t1 = dout("o_t1", [128, 512])
    o_u = dout("o_u", [128, 512])
    o_v = dout("o_v", [128, 512])
    o_w1 = dout("o_w1", [128, 512])
    o_w2 = dout("o_w2", [128, 512])
    o_bm = dout("o_bm", [1, 3])

    with tile.TileContext(nc) as tc:
        from contextlib import ExitStack

        with ExitStack() as ctx:
            persist = ctx.enter_context(tc.tile_pool(name="persist", bufs=1))
            dmaps = ctx.enter_context(
                tc.tile_pool(name="dmaps", bufs=1, space="DRAM"))
            psA = ctx.enter_context(
                tc.tile_pool(name="psA", bufs=5, space="PSUM"))
            psB = ctx.enter_context(
                tc.tile_pool(name="psB", bufs=1, space="PSUM"))

            v = nc.vector
            s = nc.scalar
            dma = nc.sync.dma_start

            # ---- persistent tiles ----
            wt = {}
            for nm, h in wq.items():
                wt[nm] = persist.tile(list(h.shape), BF16, tag="w_" + nm, name="w_" + nm)
                dma(wt[nm][:], h[:])
            bt = {}
            for nm, h in bq.items():
                n = h.shape[0]
                bt[nm] = persist.tile([n, 1], F32, tag="b_" + nm, name="b_" + nm)
                dma(bt[nm][:], h[:].rearrange("(p c) -> p c", c=1))

            ones_row = persist.tile([1, 128], F32, tag="ones_row", name="ones_row")
            v.memset(ones_row[:], 1.0)
            ones_col = persist.tile([128, 1], F32, tag="ones_col", name="ones_col")
            v.memset(ones_col[:], 1.0)
            zt = persist.tile([64, 264], BF16, tag="zt", name="zt")
            v.memset(zt[:], 0.0)

            pj_src = persist.tile([1, 16], F32, tag="pj_src", name="pj_src")
            dma(pj_src[0:1, 0:16], psc[:].rearrange("(p c) -> p c", p=1))
            ps_p = psB.tile([128, 16], F32, tag="t_bc", name="t_bc")
            nc.tensor.matmul(ps_p[:], ones_row[:], pj_src[:],
                             start=True, stop=True)
            pscal = persist.tile([128, 16], F32, tag="pscal", name="pscal")
            s.copy(pscal[:], ps_p[:])

            t1 = persist.tile([128, 512], F32, tag="t1", name="t1")
            bscal = persist.tile([128, 4], F32, tag="bscal", name="bscal")
            msrc = persist.tile([1, 4], F32, tag="msrc", name="msrc")
            v.memset(msrc[:], 0.0)

            # ---- DRAM feature maps (bf16, padded 258x258) ----
            maps = {}
            maps["zin"] = dmaps.tile([3, PADHW], BF16, tag="zin", name="zin")
            for nm in ("sfe1", "h0", "d0", "h1", "d1", "h2", "g", "g2"):
                maps[nm] = dmaps.tile([64, PADHW], BF16, tag=nm, name=nm)
            zmap = dmaps.tile([3, HW], BF16, tag="zmap", name="zmap")

            # zero the borders of maps read by 3x3 convs
            for nm in ("zin", "sfe1", "h0", "h1", "g", "g2"):
                mp = maps[nm]
                C = mp.shape[0]
                m3 = mp[:].rearrange("p (r c) -> p r c", c=WP)
                dma(mp[:, 0:WP], zt[0:C, 0:WP])
                dma(mp[:, (WP - 1) * WP : WP * WP], zt[0:C, 0:WP])
                dma(m3[:, :, 0:1],
                    zt[0:C, 0:WP].rearrange("p (r c) -> p r c", c=1))
                dma(m3[:, :, WP - 1 : WP],
                    zt[0:C, 0:WP].rearrange("p (r c) -> p r c", c=1))

            def pA(k):
                return pscal[:, k : k + 1]

            # ================= E1 + E2 (elementwise) =================
            with tc.tile_pool(name="ew", bufs=1) as ew:
                def pl(name):
                    return ew.tile([128, 512], F32, tag="pl_" + name, name="pl_" + name)

                pj = [pl(f"J{c}") for c in range(3)]
                for c in range(3):
                    dma(pj[c][:], J3[c])
                pu, pv = pl("u"), pl("v")
                pw1, pw2 = pl("w1"), pl("w2")
                dma(pu[:], u1[:]); dma(pv[:], v1[:])
                dma(pw1[:], w11[:]); dma(pw2[:], w21[:])

                js = [pl(f"js{k}") for k in range(3)]
                for k in range(3):
                    v.tensor_scalar_mul(js[k][:], pj[0][:], pA(3 * k + 0))
                    v.scalar_tensor_tensor(js[k][:], pj[1][:], pA(3 * k + 1),
                                           js[k][:], ALU.mult, ALU.add)
                    v.scalar_tensor_tensor(js[k][:], pj[2][:], pA(3 * k + 2),
                                           js[k][:], ALU.mult, ALU.add)
                jl = js[2]

                x1, x2 = pl("x1"), pl("x2")
                v.scalar_tensor_tensor(x1[:], jl[:], -1.0, pu[:],
                                       ALU.mult, ALU.add)
                v.tensor_scalar_add(x1[:], x1[:], pA(9))
                v.scalar_tensor_tensor(x2[:], jl[:], -1.0, pv[:],
                                       ALU.mult, ALU.add)
                v.tensor_scalar_add(x2[:], x2[:], pA(9))

                tmp, tmp2 = pl("tmp"), pl("tmp2")
                # u_new / v_new (soft threshold)
                v.tensor_scalar(tmp[:], x1[:], 1.0 / lam1, -1.0 / lam1,
                                ALU.min, ALU.max)
                v.tensor_sub(tmp[:], x1[:], tmp[:])
                dma(o_u[:], tmp[:])
                v.tensor_scalar(tmp[:], x2[:], 1.0 / lam2, -1.0 / lam2,
                                ALU.min, ALU.max)
                v.tensor_sub(tmp[:], x2[:], tmp[:])
                dma(o_v[:], tmp[:])
                # w_new (clip)
                v.tensor_scalar(tmp[:], x1[:], lam1, 1.0, ALU.mult, ALU.min)
                v.tensor_scalar_max(tmp[:], tmp[:], -1.0)
                dma(o_w1[:], tmp[:])
                v.tensor_scalar(tmp[:], x2[:], lam2, 1.0, ALU.mult, ALU.min)
                v.tensor_scalar_max(tmp[:], tmp[:], -1.0)
                dma(o_w2[:], tmp[:])

                # new sorted-channel values
                jm_n, js_n = pl("jm_n"), pl("js_n")
                v.scalar_tensor_tensor(tmp[:], pw1[:], -1.0 / lam1, x1[:],
                                       ALU.mult, ALU.add)
                v.tensor_mul(tmp[:], tmp[:], jl[:])
                v.tensor_add(jm_n[:], tmp[:], js[1][:])
                v.scalar_tensor_tensor(tmp[:], pw2[:], -1.0 / lam2, x2[:],
                                       ALU.mult, ALU.add)
                v.tensor_mul(tmp[:], tmp[:], jl[:])
                v.tensor_add(js_n[:], tmp[:], js[0][:])

                comp = [js_n, jm_n, jl]
                jsc = [pl(f"jsc{c}") for c in range(3)]
                for c in range(3):
                    v.tensor_scalar_mul(jsc[c][:], comp[0][:], pA(0 + c))
                    v.scalar_tensor_tensor(jsc[c][:], comp[1][:], pA(3 + c),
                                           jsc[c][:], ALU.mult, ALU.add)
                    v.scalar_tensor_tensor(jsc[c][:], comp[2][:], pA(6 + c),
                                           jsc[c][:], ALU.mult, ALU.add)

                # ---- E2 ----
                pi = [pl(f"I{c}") for c in range(3)]
                pt = [pl(f"t{c}") for c in range(3)]
                for c in range(3):
                    dma(pi[c][:], I3[c]); dma(pt[c][:], t3[c])
                for c in range(3):
                    pbp = pl(f"bp{c}")
                    dma(pbp[:], bp3[c])
                    ta = pl(f"e2a{c}")
                    tb = pl(f"e2b{c}")
                    omt = pl(f"omt{c}")
                    v.tensor_scalar(omt[:], pt[c][:], -1.0, 1.0,
                                    ALU.mult, ALU.add)
                    v.tensor_mul(ta[:], jsc[c][:], pt[c][:])
                    v.tensor_sub(ta[:], ta[:], pi[c][:])
                    v.tensor_mul(ta[:], ta[:], omt[:])
                    v.scalar_tensor_tensor(ta[:], pbp[:], GAMMA_1, ta[:],
                                           ALU.mult, ALU.subtract)
                    v.tensor_mul(tb[:], omt[:], omt[:])
                    v.tensor_scalar_add(tb[:], tb[:], GAMMA_1)
                    rcp = pl(f"rcp{c}")
                    v.reciprocal(rcp[:], tb[:])
                    v.tensor_mul(ta[:], ta[:], rcp[:])
                    prt = pl(f"prt{c}")
                    v.tensor_reduce(prt[:, 0:1], ta[:],
                                    mybir.AxisListType.X, ALU.add)
                    ps_m = psB.tile([1, 1], F32, tag="t_m", name="t_m")
                    nc.tensor.matmul(ps_m[:], prt[:, 0:1], ones_col[:],
                                     start=True, stop=True)
                    s.mul(msrc[0:1, c : c + 1], ps_m[:], 1.0 / HW)
                ps_b = psB.tile([128, 4], F32, tag="t_bc2", name="t_bc2")
                nc.tensor.matmul(ps_b[:], ones_row[:], msrc[:],
                                 start=True, stop=True)
                s.copy(bscal[:], ps_b[:])
                dma(o_bm[:], msrc[0:1, 0:3])

                tt = [pl(f"tt{c}") for c in range(3)]
                prcs = [pl(f"Rc{c}") for c in range(3)]
                for c in range(3):
                    pzc = pl(f"Zc{c}")
                    dma(pzc[:], Z3[c]); dma(prcs[c][:], R3[c])
                    ta = pl(f"e2c{c}")
                    tb = pl(f"e2d{c}")
                    jb = pl(f"jb{c}")
                    v.tensor_scalar_sub(jb[:], jsc[c][:],
                                        bscal[:, c : c + 1])
                    v.tensor_scalar(ta[:], pi[c][:], -1.0,
                                    bscal[:, c : c + 1], ALU.mult, ALU.add)
                    v.tensor_mul(ta[:], ta[:], jb[:])
                    v.scalar_tensor_tensor(tb[:], pzc[:], eta, prcs[c][:],
                                           ALU.mult, ALU.subtract)
                    ptp = pl(f"tpc{c}")
                    dma(ptp[:], tp3[c])
                    v.scalar_tensor_tensor(tb[:], ptp[:], GAMMA_2, tb[:],
                                           ALU.mult, ALU.add)
                    v.tensor_sub(tb[:], tb[:], ta[:])
                    v.tensor_mul(ta[:], jb[:], jb[:])
                    v.tensor_scalar_add(ta[:], ta[:], GAMMA_2 + eta)
                    rcp2 = pl(f"rcp2{c}")
                    v.reciprocal(rcp2[:], ta[:])
                    v.tensor_mul(tt[c][:], tb[:], rcp2[:])

                v.tensor_scalar_mul(tmp[:], tt[0][:], wt0)
                v.scalar_tensor_tensor(tmp[:], tt[1][:], wt1, tmp[:],
                                       ALU.mult, ALU.add)
                v.scalar_tensor_tensor(tmp[:], tt[2][:], wt2, tmp[:],
                                       ALU.mult, ALU.add)
                s.activation(t1[:], tmp[:], ACT.Relu)
                dma(o_t1[:], t1[:])

                rdj = pl("rdj")
                v.tensor_mul(tmp[:], t1[:], t1[:])
                v.tensor_scalar_add(tmp[:], tmp[:], beta)
                v.reciprocal(rdj[:], tmp[:])
                omt1 = pl("omt1")
                v.tensor_scalar(omt1[:], t1[:], -1.0, 1.0, ALU.mult, ALU.add)

                for c in range(3):
                    py = pl(f"Yc{c}")
                    pq = pl(f"Qc{c}")
                    dma(py[:], Y3[c]); dma(pq[:], Q3[c])
                    ta = pl(f"e2e{c}")
                    tb = pl(f"e2f{c}")
                    v.tensor_scalar_mul(ta[:], omt1[:], bscal[:, c : c + 1])
                    v.tensor_sub(ta[:], ta[:], pi[c][:])
                    v.tensor_mul(ta[:], ta[:], t1[:])
                    v.scalar_tensor_tensor(tb[:], py[:], beta, pq[:],
                                           ALU.mult, ALU.subtract)
                    v.tensor_sub(tb[:], tb[:], ta[:])
                    jn = pl(f"jn{c}")
                    v.tensor_mul(jn[:], tb[:], rdj[:])
                    dma(o_j[c], jn[:])
                    v.tensor_sub(ta[:], jn[:], py[:])
                    v.scalar_tensor_tensor(ta[:], ta[:], beta, pq[:],
                                           ALU.mult, ALU.add)
                    dma(o_q[c], ta[:])
                    # Zin
                    v.scalar_tensor_tensor(tb[:], prcs[c][:], 1.0 / eta,
                                           t1[:], ALU.mult, ALU.add)
                    zb = ew.tile([128, 512], BF16, tag=f"pl_zinbf{c}",
                                 name=f"pl_zinbf{c}")
                    zb_b = zb[:]
                    v.tensor_copy(zb_b, tb[:])
                    dst = maps["zin"][c : c + 1, :].rearrange(
                        "a (r c) -> a r c", c=WP)[0:1, 1:257, 1:257]
                    dst = dst.rearrange("a (p r) c -> (a p) r c", p=128)
                    src = zb_b.rearrange("p (r c) -> p r c", c=256)
                    dma(dst, src)

            # ================= RDN conv stack =================
            CIN_COLS = (R + 2) * WP + 12  # 8784
            with tc.tile_pool(name="cin", bufs=2) as cin, \
                 tc.tile_pool(name="cout", bufs=2) as cout, \
                 tc.tile_pool(name="cres", bufs=2) as cres:

                def tiles_of_strip():
                    q0 = 0
                    out = []
                    while q0 < SLEN:
                        out.append((q0, min(512, SLEN - q0)))
                        q0 += 512
                    return out

                def store_interior(src_buf, dst_map, y0, Cn):
                    src = src_buf[:].rearrange("p (r c) -> p r c", c=WP)
                    src = src[:, :, 1:257]
                    dst = dst_map[:].rearrange("p (r c) -> p r c", c=WP)
                    dst = dst[:, y0 + 1 : y0 + 1 + R, 1:257]
                    dma(dst, src)

                def conv3x3(src_map, dst_map, wp_t, ws_t, bias_t, relu,
                            resid_map=None):
                    M = 64
                    for st in range(NSTRIP):
                        y0 = st * R
                        it = cin.tile([128, CIN_COLS], BF16, tag="cin", name="cin")
                        dma(it[0:64, 1 : 1 + (R + 2) * WP],
                            src_map[:, y0 * WP : (y0 + R + 2) * WP])
                        dma(it[64:128, 1 : 1 + (R + 1) * WP],
                            it[0:64, 1 + WP : 1 + (R + 2) * WP])
                        ob = cout.tile([64, SLEN], BF16, tag="cout", name="cout")
                        if resid_map is not None:
                            rs = cres.tile([64, SLEN], BF16, tag="cres", name="cres")
                            dma(rs[:], resid_map[:,
                                (y0 + 1) * WP : (y0 + 1) * WP + SLEN])
                        tl = tiles_of_strip()
                        for g0 in range(0, len(tl), 4):
                            grp = tl[g0 : g0 + 4]
                            pss = [psA.tile([64, n], F32, tag="ps",
                                            name="ps") for (_, n) in grp]
                            for j in range(3):
                                for (q0, n), ps in zip(grp, pss):
                                    nc.tensor.matmul(
                                        ps[:], wp_t[:, j * M : (j + 1) * M],
                                        it[0:128, q0 + j : q0 + j + n],
                                        start=(j == 0), stop=False)
                            for j in range(3):
                                for (q0, n), ps in zip(grp, pss):
                                    nc.tensor.matmul(
                                        ps[:],
                                        ws_t[0:64, j * M : (j + 1) * M],
                                        it[0:64,
                                           q0 + 516 + j : q0 + 516 + j + n],
                                        start=False, stop=(j == 2))
                            for (q0, n), ps in zip(grp, pss):
                                obs = ob[:, q0 : q0 + n]
                                s.activation(
                                    obs, ps[:],
                                    ACT.Relu if relu else ACT.Identity,
                                    bias=bias_t[:])
                                if resid_map is not None:
                                    v.tensor_add(obs, obs,
                                                 rs[:, q0 : q0 + n])
                        store_interior(ob, dst_map, y0, 64)

                def conv1x1(a_map, b_map, dst_map, w_t, bias_t, resid):
                    for st in range(NSTRIP):
                        y0 = st * R
                        it = cin.tile([128, CIN_COLS], BF16, tag="cin", name="cin")
                        dma(it[0:64, 0:SLEN],
                            a_map[:, (y0 + 1) * WP : (y0 + 1) * WP + SLEN])
                        dma(it[64:128, 0:SLEN],
                            b_map[:, (y0 + 1) * WP : (y0 + 1) * WP + SLEN])
                        ob = cout.tile([64, SLEN], BF16, tag="cout", name="cout")
                        for (q0, n) in tiles_of_strip():
                            ps = psA.tile([64, n], F32, tag="ps", name="ps")
                            nc.tensor.matmul(ps[:], w_t[:],
                                             it[0:128, q0 : q0 + n],
                                             start=True, stop=True)
                            obs = ob[:, q0 : q0 + n]
                            s.activation(obs, ps[:], ACT.Identity,
                                         bias=bias_t[:])
                            if resid:
                                v.tensor_add(obs, obs, it[0:64, q0 : q0 + n])
                        store_interior(ob, dst_map, y0, 64)

                def conv_sfe1(src_map, dst_map, w_t, bias_t):
                    D = 2
                    for st in range(NSTRIP):
                        y0 = st * R
                        it = cin.tile([27, CIN_COLS], BF16, tag="cin9", name="cin9")
                        for a in range(3):
                            for b in range(3):
                                tk = a * 3 + b
                                src0 = y0 * WP + a * WP + b - 1
                                L = min(SLEN + 8, PADHW - max(src0, 0))
                                d0_ = D + (max(src0, 0) - src0)
                                dma(it[3 * tk : 3 * tk + 3, d0_ : d0_ + L],
                                    src_map[:, max(src0, 0) : max(src0, 0) + L])
                        ob = cout.tile([64, SLEN], BF16, tag="cout", name="cout")
                        for (q0, n) in tiles_of_strip():
                            ps = psA.tile([64, n], F32, tag="ps", name="ps")
                            nc.tensor.matmul(ps[:], w_t[:],
                                             it[0:27, D + q0 : D + q0 + n],
                                             start=True, stop=True)
                            s.activation(ob[:, q0 : q0 + n], ps[:],
                                         ACT.Identity, bias=bias_t[:])
                        store_interior(ob, dst_map, y0, 64)

                def conv_out(src_map, wp_t, ws_t, bias_t):
                    for st in range(NSTRIP):
                        y0 = st * R
                        it = cin.tile([128, CIN_COLS], BF16, tag="cin", name="cin")
                        dma(it[0:64, 1 : 1 + (R + 2) * WP],
                            src_map[:, y0 * WP : (y0 + R + 2) * WP])
                        dma(it[64:128, 1 : 1 + (R + 1) * WP],
                            it[0:64, 1 + WP : 1 + (R + 2) * WP])
                        zb = cres.tile([3, SLEN], BF16, tag="zbuf", name="zbuf", bufs=1)
                        tl = tiles_of_strip()
                        for g0 in range(0, len(tl), 4):
                            grp = tl[g0 : g0 + 4]
                            pss = [psA.tile([3, n], F32, tag="ps",
                                            name="ps") for (_, n) in grp]
                            for j in range(3):
                                for (q0, n), ps in zip(grp, pss):
                                    nc.tensor.matmul(
                                        ps[:], wp_t[:, j * 3 : (j + 1) * 3],
                                        it[0:128, q0 + j : q0 + j + n],
                                        start=(j == 0), stop=False)
                            for j in range(3):
                                for (q0, n), ps in zip(grp, pss):
                                    nc.tensor.matmul(
                                        ps[:],
                                        ws_t[0:64, j * 3 : (j + 1) * 3],
                                        it[0:64,
                                           q0 + 516 + j : q0 + 516 + j + n],
                                        start=False, stop=(j == 2))
                            for (q0, n), ps in zip(grp, pss):
                                s.activation(zb[:, q0 : q0 + n], ps[:],
                                             ACT.Identity, bias=bias_t[:])
                        src = zb[:].rearrange("p (r c) -> p r c", c=WP)
                        src = src[:, :, 1:257]
                        dst = zmap[:].rearrange("p (r c) -> p r c", c=256)
                        dst = dst[:, y0 : y0 + R, :]
                        dma(dst, src)

                conv_sfe1(maps["zin"], maps["sfe1"], wt["sfe1"], bt["sfe1"])
                conv3x3(maps["sfe1"], maps["h0"], wt["sfe2_p"], wt["sfe2_s"],
                        bt["sfe2"], relu=False)
                conv3x3(maps["h0"], maps["d0"], wt["d0_p"], wt["d0_s"],
                        bt["d0"], relu=True)
                conv1x1(maps["h0"], maps["d0"], maps["h1"], wt["l0"],
                        bt["l0"], resid=True)
                conv3x3(maps["h1"], maps["d1"], wt["d1_p"], wt["d1_s"],
                        bt["d1"], relu=True)
                conv1x1(maps["h1"], maps["d1"], maps["h2"], wt["l1"],
                        bt["l1"], resid=True)
                conv1x1(maps["h1"], maps["h2"], maps["g"], wt["g1"],
                        bt["g1"], resid=False)
                conv3x3(maps["g"], maps["g2"], wt["g2_p"], wt["g2_s"],
                        bt["g2"], relu=False, resid_map=maps["sfe1"])
                conv_out(maps["g2"], wt["out_p"], wt["out_s"], bt["out"])

            # ================= E3 =================
            with tc.tile_pool(name="e3", bufs=2) as e3:
                for c in range(3):
                    rp = e3.tile([128, 512], F32, tag="rp", name="rp")
                    dma(rp[:], R3[c])
                    zpb = e3.tile([128, 512], BF16, tag="zpb", name="zpb")
                    dma(zpb[:], zmap[:].rearrange(
                        "a (p c) -> a p c", c=512)[c])
                    zp = e3.tile([128, 512], F32, tag="zp", name="zp")
                    v.tensor_copy(zp[:], zpb[:])
                    dma(o_z[c], zp[:])
                    rn = e3.tile([128, 512], F32, tag="rn", name="rn")
                    v.tensor_sub(rn[:], t1[:], zp[:])
                    v.scalar_tensor_tensor(rn[:], rn[:], eta, rp[:],
                                           ALU.mult, ALU.add)
                    dma(o_r[c], rn[:])

    nc.finalize()
    _split_sync_waits(nc, max_waits=1)
    return nc


_CACHE = {}


def _pack_weights(params):
    def bf(x):
        return np.ascontiguousarray(x.astype(_BF))

    out = {}
    for nm, key in (("sfe2", "sfe2_w"), ("d0", "rdb0_dense_w"),
                    ("d1", "rdb1_dense_w"), ("g2", "gff2_w")):
        Wt = np.asarray(params[key], np.float32)  # [64,64,3,3]
        wp = np.zeros((128, 192), np.float32)
        ws = np.zeros((64, 192), np.float32)
        for j in range(3):
            wp[0:64, j * 64 : (j + 1) * 64] = Wt[:, :, 0, j].T
            wp[64:128, j * 64 : (j + 1) * 64] = Wt[:, :, 1, j].T
            ws[0:64, j * 64 : (j + 1) * 64] = Wt[:, :, 2, j].T
        out["w_" + nm + "_p"] = bf(wp)
        out["w_" + nm + "_s"] = bf(ws)
    Wt = np.asarray(params["out_w"], np.float32)  # [3,64,3,3]
    wp = np.zeros((128, 9), np.float32)
    ws = np.zeros((64, 9), np.float32)
    for j in range(3):
        wp[0:64, j * 3 : (j + 1) * 3] = Wt[:, :, 0, j].T
        wp[64:128, j * 3 : (j + 1) * 3] = Wt[:, :, 1, j].T
        ws[0:64, j * 3 : (j + 1) * 3] = Wt[:, :, 2, j].T
    out["w_out_p"] = bf(wp)
    out["w_out_s"] = bf(ws)
    Wt = np.asarray(params["sfe1_w"], np.float32)  # [64,3,3,3]
    w9 = np.zeros((27, 64), np.float32)
    for a in range(3):
        for b in range(3):
            tk = a * 3 + b
            w9[3 * tk : 3 * tk + 3, :] = Wt[:, :, a, b].T
    out["w_sfe1"] = bf(w9)
    for nm, key in (("l0", "rdb0_lff_w"), ("l1", "rdb1_lff_w"),
                    ("g1", "gff1_w")):
        Wt = np.asarray(params[key], np.float32)  # [64,128,1,1]
        out["w_" + nm] = bf(Wt[:, :, 0, 0].T)
    for nm, key in (("sfe1", "sfe1_b"), ("sfe2", "sfe2_b"),
                    ("d0", "rdb0_dense_b"), ("l0", "rdb0_lff_b"),
                    ("d1", "rdb1_dense_b"), ("l1", "rdb1_lff_b"),
                    ("g1", "gff1_b"), ("g2", "gff2_b"), ("out", "out_b")):
        out["b_" + nm] = np.ascontiguousarray(
            np.asarray(params[key], np.float32))
    return out


def kernel(**inputs):
    params = inputs["params"]
    beta = float(np.asarray(params["beta"])[0])
    eta = float(np.asarray(params["eta"])[0])
    lam1 = float(np.asarray(params["lambda_1"])[0])
    lam2 = float(np.asarray(params["lambda_2"])[0])
    wtd = np.asarray(params["t1d_w"], np.float32)[0, :, 0, 0]
    sc = dict(beta=beta, eta=eta, lam1=lam1, lam2=lam2,
              wt0=float(wtd[0]), wt1=float(wtd[1]), wt2=float(wtd[2]))
    key = tuple(sorted(sc.items()))
    if key not in _CACHE:
        _CACHE[key] = _build_program(sc)
    nc = _CACHE[key]

    wpk = _pack_weights(params)

    f32 = lambda x: np.ascontiguousarray(np.asarray(x, np.float32))
    J = f32(inputs["J"])
    means = J.mean(axis=(2, 3), dtype=np.float32)  # [8,3]
    idx = np.argsort(means, axis=1, kind="stable")

    in_maps = []
    for b in range(NCORES):
        P = np.zeros((3, 3), np.float32)
        for k in range(3):
            P[k, idx[b, k]] = 1.0
        psc = np.zeros(16, np.float32)
        psc[0:9] = P.reshape(-1)
        psc[9] = means[b, idx[b, 2]]
        m = {
            "I3": f32(inputs["I"][b]).reshape(3, 128, 512),
            "tp3": f32(inputs["t_p"][b]).reshape(3, 128, 512),
            "bp3": f32(inputs["B_p"][b]).reshape(3, 128, 512),
            "t3": f32(inputs["t"][b]).reshape(3, 128, 512),
            "J3": J[b].reshape(3, 128, 512),
            "Y3": f32(inputs["Y"][b]).reshape(3, 128, 512),
            "Z3": f32(inputs["Z"][b]).reshape(3, 128, 512),
            "Q3": f32(inputs["Q"][b]).reshape(3, 128, 512),
            "R3": f32(inputs["R"][b]).reshape(3, 128, 512),
            "u1": f32(inputs["u"][b]).reshape(128, 512),
            "v1": f32(inputs["v"][b]).reshape(128, 512),
            "w11": f32(inputs["w_1"][b]).reshape(128, 512),
            "w21": f32(inputs["w_2"][b]).reshape(128, 512),
            "psc": psc,
        }
        m.update(wpk)
        in_maps.append(m)

    trace = bool(int(os.environ.get("KERNEL_TRACE", "0")))
    tkw = {}
    if trace:
        try:
            sys.path.insert(0, "/root/problem/work")
            import profhook

            profhook.install()
            tkw = dict(trace=True, tmpdir=os.environ.get(
                "KERNEL_TRACE_DIR", "/root/problem/work/trace_out"))
        except Exception:
            tkw = {}
    r = run_bass_kernel_spmd(nc, in_maps, list(range(NCORES)), **tkw)
    res = r.results
    if trace and getattr(r, "exec_time_ns", None) is not None:
        kernel.last_exec_time_ns = r.exec_time_ns

    B = np.empty((8, 3, H, W), np.float32)
    t_new = np.empty((8, 3, H, W), np.float32)
    Jn = np.empty((8, 3, H, W), np.float32)
    Zn = np.empty((8, 3, H, W), np.float32)
    Qn = np.empty((8, 3, H, W), np.float32)
    Rn = np.empty((8, 3, H, W), np.float32)
    un = np.empty((8, 1, H, W), np.float32)
    vn = np.empty((8, 1, H, W), np.float32)
    w1n = np.empty((8, 1, H, W), np.float32)
    w2n = np.empty((8, 1, H, W), np.float32)
    for b in range(NCORES):
        o = res[b]
        B[b] = o["o_bm"].reshape(3, 1, 1)
        t_new[b] = o["o_t1"].reshape(1, H, W)
        Jn[b] = o["o_j"].reshape(3, H, W)
        Zn[b] = o["o_z"].reshape(3, H, W)
        Qn[b] = o["o_q"].reshape(3, H, W)
        Rn[b] = o["o_r"].reshape(3, H, W)
        un[b, 0] = o["o_u"].reshape(H, W)
        vn[b, 0] = o["o_v"].reshape(H, W)
        w1n[b, 0] = o["o_w1"].reshape(H, W)
        w2n[b, 0] = o["o_w2"].reshape(H, W)
    Y = f32(inputs["Y"])
    return (B, t_new, Jn, Y, Zn, Qn, Rn, un, vn, w1n, w2n,
            f32(params["beta"]))


# revision 25
# speedup vs baseline: 1.7644x; 1.3326x over previous
"""Trainium2 Bass kernel for nn_BasicBlock_1709396984498.

Data-parallel over batch: 1 sample per NeuronCore (8 cores).
Per core: elementwise ADMM phase in fp32 planes [128,512], channel
gather/scatter via host-computed one-hot permutation, RDN conv stack
streamed through padded DRAM maps in bf16 (3x3 convs as 6 matmuls per
512-px tile: 3 row-shift-paired taps at K=128 + 3 singles at K=64).
"""

import json
import os
import sys

for _p in (
    "/root/.axon_site",
    "/root/.axon_site/_ro/trn_rl_repo",
    "/root/.axon_site/_ro/pypackages",
    "/opt/trn_rl_repo",
):
    if os.path.isdir(_p) and _p not in sys.path:
        sys.path.append(_p)

import ml_dtypes
import numpy as np

import concourse.bass as bass
import concourse.bass_isa as bass_isa
import concourse.tile as tile
from concourse import mybir
import concourse.bass_utils as _bu
from concourse.bass_utils import run_bass_kernel_spmd


F32 = mybir.dt.float32
BF16 = mybir.dt.bfloat16
ALU = mybir.AluOpType
ACT = mybir.ActivationFunctionType

GAMMA_1, GAMMA_2 = 0.3, 0.7
H = W = 256
HW = H * W
WP = W + 2  # padded width (258)
PADHW = (H + 2) * WP  # 66564
G = 64
R = 32  # strip rows
NSTRIP = H // R
SLEN = R * WP  # 8256 flat output positions per strip
NCORES = 8

_BF = ml_dtypes.bfloat16


def _split_sync_waits(nc, max_waits=1):
    """Post-pass on the final BIR:
    1) Replace InstLdweights whose weights operand matches the previous
       PE weight load (with only Matmults between) by a NoOp that keeps
       the same sync_info — weight-stationary reuse the toolchain's
       disabled ldw-opt would otherwise do.
    2) Walrus in this env rejects >1 sync-wait on CTRL ops; split
       overflow waits onto preceding same-engine NoOps."""
    import bass_rust

    m = json.loads(bass_rust.module_to_json_string(nc.m))
    for f in m["functions"]:
        for bb in f["blocks"]:
            last_w = None
            for inst in bb["instructions"]:
                op = inst["opcode"]
                if inst.get("engine") != "PE":
                    continue
                if op == "Ldweights":
                    key = json.dumps(inst.get("ins"), sort_keys=True)
                    if key == last_w and not (
                        (inst.get("sync_info") or {}).get("on_update")
                    ):
                        inst["opcode"] = "NoOp"
                        inst["ins"] = []
                        inst["outs"] = []
                    else:
                        last_w = key
                elif op not in ("Matmult",):
                    last_w = None
    n = [0]
    changed = True
    for f in m["functions"]:
        for bb in f["blocks"]:
            out = []
            for inst in bb["instructions"]:
                si = inst.get("sync_info")
                waits = (si or {}).get("on_wait") or []
                if len(waits) > max_waits:
                    changed = True
                    keep = waits[len(waits) - max_waits :]
                    over = waits[: len(waits) - max_waits]
                    for i in range(0, len(over), max_waits):
                        n[0] += 1
                        out.append(
                            {
                                "debug": inst.get("debug", 0),
                                "engine": inst["engine"],
                                "ins": [],
                                "outs": [],
                                "name": f"wsplit-{n[0]}",
                                "opcode": "NoOp",
                                "sync_info": {
                                    "on_update": [],
                                    "on_wait": over[i : i + max_waits],
                                },
                            }
                        )
                    si["on_wait"] = keep
                out.append(inst)
            bb["instructions"] = out
    if changed:
        nc.m = bass_rust.module_from_json_string(json.dumps(m))
    return nc


def _build_program(sc):
    """sc: dict of python-float scalars (beta, eta, lam1, lam2, wt0..2)."""
    beta, eta = sc["beta"], sc["eta"]
    lam1, lam2 = sc["lam1"], sc["lam2"]
    wt0, wt1, wt2 = sc["wt0"], sc["wt1"], sc["wt2"]

    nc = bass.Bass("TRN2", target_bir_lowering=False, debug=False,
                   num_devices=NCORES)

    def din(name, shape, dt=F32):
        return nc.dram_tensor(name, shape, dt, kind="ExternalInput")

    def dout(name, shape, dt=F32):
        return nc.dram_tensor(name, shape, dt, kind="ExternalOutput")

    I3 = din("I3", [3, 128, 512])
    tp3 = din("tp3", [3, 128, 512])
    bp3 = din("bp3", [3, 128, 512])
    t3 = din("t3", [3, 128, 512])
    J3 = din("J3", [3, 128, 512])
    Y3 = din("Y3", [3, 128, 512])
    Z3 = din("Z3", [3, 128, 512])
    Q3 = din("Q3", [3, 128, 512])
    R3 = din("R3", [3, 128, 512])
    u1 = din("u1", [128, 512])
    v1 = din("v1", [128, 512])
    w11 = din("w11", [128, 512])
    w21 = din("w21", [128, 512])
    psc = din("psc", [16])  # P flat (9), jlbar (1), pad

    wq = {}
    for nm in ("sfe2", "d0", "d1", "g2"):
        wq[nm + "_p"] = din("w_" + nm + "_p", [128, 192], BF16)
        wq[nm + "_s"] = din("w_" + nm + "_s", [64, 192], BF16)
    wq["out_p"] = din("w_out_p", [128, 9], BF16)
    wq["out_s"] = din("w_out_s", [64, 9], BF16)
    wq["sfe1"] = din("w_sfe1", [27, 64], BF16)
    for nm in ("l0", "l1", "g1"):
        wq[nm] = din("w_" + nm, [128, 64], BF16)
    bq = {}
    for nm in ("sfe1", "sfe2", "d0", "l0", "d1", "l1", "g1", "g2"):
        bq[nm] = din("b_" + nm, [64])
    bq["out"] = din("b_out", [3])

    o_j = dout("o_j", [3, 128, 512])
    o_q = dout("o_q", [3, 128, 512])
    o_z = dout("o_z", [3, 128, 512])
    o_r = dout("o_r", [3, 128, 512])
    o_t1 = dout("o_t1", [128, 512])
    o_u = dout("o_u", [128, 512])
    o_v = dout("o_v", [128, 512])
    o_w1 = dout("o_w1", [128, 512])
    o_w2 = dout("o_w2", [128, 512])
    o_bm = dout("o_bm", [1, 3])

    with tile.TileContext(nc) as tc:
        from contextlib import ExitStack

        with ExitStack() as ctx:
            persist = ctx.enter_context(tc.tile_pool(name="persist", bufs=1))
            dmaps = ctx.enter_context(
                tc.tile_pool(name="dmaps", bufs=1, space="DRAM"))
            psA = ctx.enter_context(
                tc.tile_pool(name="psA", bufs=5, space="PSUM"))
            psB = ctx.enter_context(
                tc.tile_pool(name="psB", bufs=1, space="PSUM"))

            v = nc.vector
            s = nc.scalar
            dma = nc.sync.dma_start

            # ---- persistent tiles ----
            wt = {}
            for nm, h in wq.items():
                wt[nm] = persist.tile(list(h.shape), BF16, tag="w_" + nm, name="w_" + nm)
                dma(wt[nm][:], h[:])
            bt = {}
            for nm, h in bq.items():
                n = h.shape[0]
                bt[nm] = persist.tile([n, 1], F32, tag="b_" + nm, name="b_" + nm)
                dma(bt[nm][:], h[:].rearrange("(p c) -> p c", c=1))

            ones_row = persist.tile([1, 128], F32, tag="ones_row", name="ones_row")
            v.memset(ones_row[:], 1.0)
            ones_col = persist.tile([128, 1], F32, tag="ones_col", name="ones_col")
            v.memset(ones_col[:], 1.0)
            zt = persist.tile([64, 264], BF16, tag="zt", name="zt")
            v.memset(zt[:], 0.0)

            pj_src = persist.tile([1, 16], F32, tag="pj_src", name="pj_src")
            dma(pj_src[0:1, 0:16], psc[:].rearrange("(p c) -> p c", p=1))
            ps_p = psB.tile([128, 16], F32, tag="t_bc", name="t_bc")
            nc.tensor.matmul(ps_p[:], ones_row[:], pj_src[:],
                             start=True, stop=True)
            pscal = persist.tile([128, 16], F32, tag="pscal", name="pscal")
            s.copy(pscal[:], ps_p[:])

            t1 = persist.tile([128, 512], F32, tag="t1", name="t1")
            bscal = persist.tile([128, 4], F32, tag="bscal", name="bscal")
            msrc = persist.tile([1, 4], F32, tag="msrc", name="msrc")
            v.memset(msrc[:], 0.0)

            # ---- DRAM feature maps (bf16, padded 258x258) ----
            maps = {}
            maps["zin"] = dmaps.tile([3, PADHW], BF16, tag="zin", name="zin")
            for nm in ("sfe1", "h0", "d0", "h1", "d1", "h2", "g", "g2"):
                maps[nm] = dmaps.tile([64, PADHW], BF16, tag=nm, name=nm)
            zmap = dmaps.tile([3, HW], BF16, tag="zmap", name="zmap")

            # zero top/bottom pad rows of maps read by 3x3 convs (cheap,
            # contiguous); pad columns are zeroed in SBUF by the producer
            # (store_full), except zin whose planes store is strided.
            for nm in ("zin", "sfe1", "h0", "h1", "g", "g2"):
                mp = maps[nm]
                C = mp.shape[0]
                dma(mp[:, 0:WP], zt[0:C, 0:WP])
                dma(mp[:, (WP - 1) * WP : WP * WP], zt[0:C, 0:WP])
            m3 = maps["zin"][:].rearrange("p (r c) -> p r c", c=WP)
            dma(m3[:, :, 0:1],
                zt[0:3, 0:WP].rearrange("p (r c) -> p r c", c=1))
            dma(m3[:, :, WP - 1 : WP],
                zt[0:3, 0:WP].rearrange("p (r c) -> p r c", c=1))

            def pA(k):
                return pscal[:, k : k + 1]

            # ================= E1 + E2 (elementwise) =================
            with tc.tile_pool(name="ew", bufs=1) as ew:
                def pl(name):
                    return ew.tile([128, 512], F32, tag="pl_" + name, name="pl_" + name)

                pj = [pl(f"J{c}") for c in range(3)]
                for c in range(3):
                    dma(pj[c][:], J3[c])
                pu, pv = pl("u"), pl("v")
                pw1, pw2 = pl("w1"), pl("w2")
                dma(pu[:], u1[:]); dma(pv[:], v1[:])
                dma(pw1[:], w11[:]); dma(pw2[:], w21[:])

                js = [pl(f"js{k}") for k in range(3)]
                for k in range(3):
                    v.tensor_scalar_mul(js[k][:], pj[0][:], pA(3 * k + 0))
                    v.scalar_tensor_tensor(js[k][:], pj[1][:], pA(3 * k + 1),
                                           js[k][:], ALU.mult, ALU.add)
                    v.scalar_tensor_tensor(js[k][:], pj[2][:], pA(3 * k + 2),
                                           js[k][:], ALU.mult, ALU.add)
                jl = js[2]

                x1, x2 = pl("x1"), pl("x2")
                v.scalar_tensor_tensor(x1[:], jl[:], -1.0, pu[:],
                                       ALU.mult, ALU.add)
                v.tensor_scalar_add(x1[:], x1[:], pA(9))
                v.scalar_tensor_tensor(x2[:], jl[:], -1.0, pv[:],
                                       ALU.mult, ALU.add)
                v.tensor_scalar_add(x2[:], x2[:], pA(9))

                tmp, tmp2 = pl("tmp"), pl("tmp2")
                # u_new / v_new (soft threshold)
                v.tensor_scalar(tmp[:], x1[:], 1.0 / lam1, -1.0 / lam1,
                                ALU.min, ALU.max)
                v.tensor_sub(tmp[:], x1[:], tmp[:])
                dma(o_u[:], tmp[:])
                v.tensor_scalar(tmp[:], x2[:], 1.0 / lam2, -1.0 / lam2,
                                ALU.min, ALU.max)
                v.tensor_sub(tmp[:], x2[:], tmp[:])
                dma(o_v[:], tmp[:])
                # w_new (clip)
                v.tensor_scalar(tmp[:], x1[:], lam1, 1.0, ALU.mult, ALU.min)
                v.tensor_scalar_max(tmp[:], tmp[:], -1.0)
                dma(o_w1[:], tmp[:])
                v.tensor_scalar(tmp[:], x2[:], lam2, 1.0, ALU.mult, ALU.min)
                v.tensor_scalar_max(tmp[:], tmp[:], -1.0)
                dma(o_w2[:], tmp[:])

                # new sorted-channel values
                jm_n, js_n = pl("jm_n"), pl("js_n")
                v.scalar_tensor_tensor(tmp[:], pw1[:], -1.0 / lam1, x1[:],
                                       ALU.mult, ALU.add)
                v.tensor_mul(tmp[:], tmp[:], jl[:])
                v.tensor_add(jm_n[:], tmp[:], js[1][:])
                v.scalar_tensor_tensor(tmp[:], pw2[:], -1.0 / lam2, x2[:],
                                       ALU.mult, ALU.add)
                v.tensor_mul(tmp[:], tmp[:], jl[:])
                v.tensor_add(js_n[:], tmp[:], js[0][:])

                comp = [js_n, jm_n, jl]
                jsc = [pl(f"jsc{c}") for c in range(3)]
                for c in range(3):
                    v.tensor_scalar_mul(jsc[c][:], comp[0][:], pA(0 + c))
                    v.scalar_tensor_tensor(jsc[c][:], comp[1][:], pA(3 + c),
                                           jsc[c][:], ALU.mult, ALU.add)
                    v.scalar_tensor_tensor(jsc[c][:], comp[2][:], pA(6 + c),
                                           jsc[c][:], ALU.mult, ALU.add)

                # ---- E2 ----
                pi = [pl(f"I{c}") for c in range(3)]
                pt = [pl(f"t{c}") for c in range(3)]
                for c in range(3):
                    dma(pi[c][:], I3[c]); dma(pt[c][:], t3[c])
                for c in range(3):
                    pbp = pl(f"bp{c}")
                    dma(pbp[:], bp3[c])
                    ta = pl(f"e2a{c}")
                    tb = pl(f"e2b{c}")
                    omt = pl(f"omt{c}")
                    v.tensor_scalar(omt[:], pt[c][:], -1.0, 1.0,
                                    ALU.mult, ALU.add)
                    v.tensor_mul(ta[:], jsc[c][:], pt[c][:])
                    v.tensor_sub(ta[:], ta[:], pi[c][:])
                    v.tensor_mul(ta[:], ta[:], omt[:])
                    v.scalar_tensor_tensor(ta[:], pbp[:], GAMMA_1, ta[:],
                                           ALU.mult, ALU.subtract)
                    v.tensor_mul(tb[:], omt[:], omt[:])
                    v.tensor_scalar_add(tb[:], tb[:], GAMMA_1)
                    rcp = pl(f"rcp{c}")
                    v.reciprocal(rcp[:], tb[:])
                    v.tensor_mul(ta[:], ta[:], rcp[:])
                    prt = pl(f"prt{c}")
                    v.tensor_reduce(prt[:, 0:1], ta[:],
                                    mybir.AxisListType.X, ALU.add)
                    ps_m = psB.tile([1, 1], F32, tag="t_m", name="t_m")
                    nc.tensor.matmul(ps_m[:], prt[:, 0:1], ones_col[:],
                                     start=True, stop=True)
                    s.mul(msrc[0:1, c : c + 1], ps_m[:], 1.0 / HW)
                ps_b = psB.tile([128, 4], F32, tag="t_bc2", name="t_bc2")
                nc.tensor.matmul(ps_b[:], ones_row[:], msrc[:],
                                 start=True, stop=True)
                s.copy(bscal[:], ps_b[:])
                dma(o_bm[:], msrc[0:1, 0:3])

                tt = [pl(f"tt{c}") for c in range(3)]
                prcs = [pl(f"Rc{c}") for c in range(3)]
                for c in range(3):
                    pzc = pl(f"Zc{c}")
                    dma(pzc[:], Z3[c]); dma(prcs[c][:], R3[c])
                    ta = pl(f"e2c{c}")
                    tb = pl(f"e2d{c}")
                    jb = pl(f"jb{c}")
                    v.tensor_scalar_sub(jb[:], jsc[c][:],
                                        bscal[:, c : c + 1])
                    v.tensor_scalar(ta[:], pi[c][:], -1.0,
                                    bscal[:, c : c + 1], ALU.mult, ALU.add)
                    v.tensor_mul(ta[:], ta[:], jb[:])
                    v.scalar_tensor_tensor(tb[:], pzc[:], eta, prcs[c][:],
                                           ALU.mult, ALU.subtract)
                    ptp = pl(f"tpc{c}")
                    dma(ptp[:], tp3[c])
                    v.scalar_tensor_tensor(tb[:], ptp[:], GAMMA_2, tb[:],
                                           ALU.mult, ALU.add)
                    v.tensor_sub(tb[:], tb[:], ta[:])
                    v.tensor_mul(ta[:], jb[:], jb[:])
                    v.tensor_scalar_add(ta[:], ta[:], GAMMA_2 + eta)
                    rcp2 = pl(f"rcp2{c}")
                    v.reciprocal(rcp2[:], ta[:])
                    v.tensor_mul(tt[c][:], tb[:], rcp2[:])

                v.tensor_scalar_mul(tmp[:], tt[0][:], wt0)
                v.scalar_tensor_tensor(tmp[:], tt[1][:], wt1, tmp[:],
                                       ALU.mult, ALU.add)
                v.scalar_tensor_tensor(tmp[:], tt[2][:], wt2, tmp[:],
                                       ALU.mult, ALU.add)
                s.activation(t1[:], tmp[:], ACT.Relu)
                dma(o_t1[:], t1[:])

                rdj = pl("rdj")
                v.tensor_mul(tmp[:], t1[:], t1[:])
                v.tensor_scalar_add(tmp[:], tmp[:], beta)
                v.reciprocal(rdj[:], tmp[:])
                omt1 = pl("omt1")
                v.tensor_scalar(omt1[:], t1[:], -1.0, 1.0, ALU.mult, ALU.add)

                for c in range(3):
                    py = pl(f"Yc{c}")
                    pq = pl(f"Qc{c}")
                    dma(py[:], Y3[c]); dma(pq[:], Q3[c])
                    ta = pl(f"e2e{c}")
                    tb = pl(f"e2f{c}")
                    v.tensor_scalar_mul(ta[:], omt1[:], bscal[:, c : c + 1])
                    v.tensor_sub(ta[:], ta[:], pi[c][:])
                    v.tensor_mul(ta[:], ta[:], t1[:])
                    v.scalar_tensor_tensor(tb[:], py[:], beta, pq[:],
                                           ALU.mult, ALU.subtract)
                    v.tensor_sub(tb[:], tb[:], ta[:])
                    jn = pl(f"jn{c}")
                    v.tensor_mul(jn[:], tb[:], rdj[:])
                    dma(o_j[c], jn[:])
                    v.tensor_sub(ta[:], jn[:], py[:])
                    v.scalar_tensor_tensor(ta[:], ta[:], beta, pq[:],
                                           ALU.mult, ALU.add)
                    dma(o_q[c], ta[:])
                    # Zin
                    v.scalar_tensor_tensor(tb[:], prcs[c][:], 1.0 / eta,
                                           t1[:], ALU.mult, ALU.add)
                    zb = ew.tile([128, 512], BF16, tag=f"pl_zinbf{c}",
                                 name=f"pl_zinbf{c}")
                    zb_b = zb[:]
                    v.tensor_copy(zb_b, tb[:])
                    dst = maps["zin"][c : c + 1, :].rearrange(
                        "a (r c) -> a r c", c=WP)[0:1, 1:257, 1:257]
                    dst = dst.rearrange("a (p r) c -> (a p) r c", p=128)
                    src = zb_b.rearrange("p (r c) -> p r c", c=256)
                    dma(dst, src)

            # ================= RDN conv stack =================
            CIN_COLS = (R + 2) * WP + 12  # 8784
            with tc.tile_pool(name="cin", bufs=2) as cin, \
                 tc.tile_pool(name="cout", bufs=2) as cout, \
                 tc.tile_pool(name="cres", bufs=2) as cres:

                def tiles_of_strip():
                    q0 = 0
                    out = []
                    while q0 < SLEN:
                        out.append((q0, min(512, SLEN - q0)))
                        q0 += 512
                    return out

                def store_interior(src_buf, dst_map, y0, Cn):
                    s3 = src_buf[:].rearrange("p (r c) -> p r c", c=WP)
                    v.memset(s3[:, :, 0:1], 0.0)
                    v.memset(s3[:, :, WP - 1 : WP], 0.0)
                    dma(dst_map[:, (y0 + 1) * WP : (y0 + 1 + R) * WP],
                        src_buf[:])

                def conv3x3(src_map, dst_map, wp_t, ws_t, bias_t, relu,
                            resid_map=None):
                    M = 64
                    for st in range(NSTRIP):
                        y0 = st * R
                        it = cin.tile([128, CIN_COLS], BF16, tag="cin", name="cin")
                        dma(it[0:64, 1 : 1 + (R + 2) * WP],
                            src_map[:, y0 * WP : (y0 + R + 2) * WP])
                        dma(it[64:128, 1 : 1 + (R + 1) * WP],
                            it[0:64, 1 + WP : 1 + (R + 2) * WP])
                        ob = cout.tile([64, SLEN], BF16, tag="cout", name="cout")
                        if resid_map is not None:
                            rs = cres.tile([64, SLEN], BF16, tag="cres", name="cres")
                            dma(rs[:], resid_map[:,
                                (y0 + 1) * WP : (y0 + 1) * WP + SLEN])
                        tl = tiles_of_strip()
                        for g0 in range(0, len(tl), 4):
                            grp = tl[g0 : g0 + 4]
                            pss = [psA.tile([64, n], F32, tag="ps",
                                            name="ps") for (_, n) in grp]
                            for j in range(3):
                                for (q0, n), ps in zip(grp, pss):
                                    nc.tensor.matmul(
                                        ps[:], wp_t[:, j * M : (j + 1) * M],
                                        it[0:128, q0 + j : q0 + j + n],
                                        start=(j == 0), stop=False)
                            for j in range(3):
                                for (q0, n), ps in zip(grp, pss):
                                    nc.tensor.matmul(
                                        ps[:],
                                        ws_t[0:64, j * M : (j + 1) * M],
                                        it[0:64,
                                           q0 + 516 + j : q0 + 516 + j + n],
                                        start=False, stop=(j == 2))
                            for (q0, n), ps in zip(grp, pss):
                                obs = ob[:, q0 : q0 + n]
                                s.activation(
                                    obs, ps[:],
                                    ACT.Relu if relu else ACT.Identity,
                                    bias=bias_t[:])
                                if resid_map is not None:
                                    v.tensor_add(obs, obs,
                                                 rs[:, q0 : q0 + n])
                        store_interior(ob, dst_map, y0, 64)

                def conv1x1(a_map, b_map, dst_map, w_t, bias_t, resid):
                    for st in range(NSTRIP):
                        y0 = st * R
                        it = cin.tile([128, CIN_COLS], BF16, tag="cin", name="cin")
                        dma(it[0:64, 0:SLEN],
                            a_map[:, (y0 + 1) * WP : (y0 + 1) * WP + SLEN])
                        dma(it[64:128, 0:SLEN],
                            b_map[:, (y0 + 1) * WP : (y0 + 1) * WP + SLEN])
                        ob = cout.tile([64, SLEN], BF16, tag="cout", name="cout")
                        for (q0, n) in tiles_of_strip():
                            ps = psA.tile([64, n], F32, tag="ps", name="ps")
                            nc.tensor.matmul(ps[:], w_t[:],
                                             it[0:128, q0 : q0 + n],
                                             start=True, stop=True)
                            obs = ob[:, q0 : q0 + n]
                            s.activation(obs, ps[:], ACT.Identity,
                                         bias=bias_t[:])
                            if resid:
                                v.tensor_add(obs, obs, it[0:64, q0 : q0 + n])
                        store_interior(ob, dst_map, y0, 64)

                def conv_sfe1(src_map, dst_map, w_t, bias_t):
                    D = 2
                    for st in range(NSTRIP):
                        y0 = st * R
                        it = cin.tile([27, CIN_COLS], BF16, tag="cin9", name="cin9")
                        for a in range(3):
                            for b in range(3):
                                tk = a * 3 + b
                                src0 = y0 * WP + a * WP + b - 1
                                L = min(SLEN + 8, PADHW - max(src0, 0))
                                d0_ = D + (max(src0, 0) - src0)
                                dma(it[3 * tk : 3 * tk + 3, d0_ : d0_ + L],
                                    src_map[:, max(src0, 0) : max(src0, 0) + L])
                        ob = cout.tile([64, SLEN], BF16, tag="cout", name="cout")
                        for (q0, n) in tiles_of_strip():
                            ps = psA.tile([64, n], F32, tag="ps", name="ps")
                            nc.tensor.matmul(ps[:], w_t[:],
                                             it[0:27, D + q0 : D + q0 + n],
                                             start=True, stop=True)
                            s.activation(ob[:, q0 : q0 + n], ps[:],
                                         ACT.Identity, bias=bias_t[:])
                        store_interior(ob, dst_map, y0, 64)

                def conv_out(src_map, wp_t, ws_t, bias_t):
                    for st in range(NSTRIP):
                        y0 = st * R
                        it = cin.tile([128, CIN_COLS], BF16, tag="cin", name="cin")
                        dma(it[0:64, 1 : 1 + (R + 2) * WP],
                            src_map[:, y0 * WP : (y0 + R + 2) * WP])
                        dma(it[64:128, 1 : 1 + (R + 1) * WP],
                            it[0:64, 1 + WP : 1 + (R + 2) * WP])
                        zb = cres.tile([3, SLEN], BF16, tag="zbuf", name="zbuf", bufs=1)
                        tl = tiles_of_strip()
                        for g0 in range(0, len(tl), 4):
                            grp = tl[g0 : g0 + 4]
                            pss = [psA.tile([3, n], F32, tag="ps",
                                            name="ps") for (_, n) in grp]
                            for j in range(3):
                                for (q0, n), ps in zip(grp, pss):
                                    nc.tensor.matmul(
                                        ps[:], wp_t[:, j * 3 : (j + 1) * 3],
                                        it[0:128, q0 + j : q0 + j + n],
                                        start=(j == 0), stop=False)
                            for j in range(3):
                                for (q0, n), ps in zip(grp, pss):
                                    nc.tensor.matmul(
                                        ps[:],
                                        ws_t[0:64, j * 3 : (j + 1) * 3],
                                        it[0:64,
                                           q0 + 516 + j : q0 + 516 + j + n],
                                        start=False, stop=(j == 2))
                            for (q0, n), ps in zip(grp, pss):
                                s.activation(zb[:, q0 : q0 + n], ps[:],
                                             ACT.Identity, bias=bias_t[:])
                        src = zb[:].rearrange("p (r c) -> p r c", c=WP)
                        src = src[:, :, 1:257]
                        dst = zmap[:].rearrange("p (r c) -> p r c", c=256)
                        dst = dst[:, y0 : y0 + R, :]
                        dma(dst, src)

                conv_sfe1(maps["zin"], maps["sfe1"], wt["sfe1"], bt["sfe1"])
                conv3x3(maps["sfe1"], maps["h0"], wt["sfe2_p"], wt["sfe2_s"],
                        bt["sfe2"], relu=False)
                conv3x3(maps["h0"], maps["d0"], wt["d0_p"], wt["d0_s"],
                        bt["d0"], relu=True)
                conv1x1(maps["h0"], maps["d0"], maps["h1"], wt["l0"],
                        bt["l0"], resid=True)
                conv3x3(maps["h1"], maps["d1"], wt["d1_p"], wt["d1_s"],
                        bt["d1"], relu=True)
                conv1x1(maps["h1"], maps["d1"], maps["h2"], wt["l1"],
                        bt["l1"], resid=True)
                conv1x1(maps["h1"], maps["h2"], maps["g"], wt["g1"],
                        bt["g1"], resid=False)
                conv3x3(maps["g"], maps["g2"], wt["g2_p"], wt["g2_s"],
                        bt["g2"], relu=False, resid_map=maps["sfe1"])
                conv_out(maps["g2"], wt["out_p"], wt["out_s"], bt["out"])

            # ================= E3 =================
            with tc.tile_pool(name="e3", bufs=2) as e3:
                for c in range(3):
                    rp = e3.tile([128, 512], F32, tag="rp", name="rp")
                    dma(rp[:], R3[c])
                    zpb = e3.tile([128, 512], BF16, tag="zpb", name="zpb")
                    dma(zpb[:], zmap[:].rearrange(
                        "a (p c) -> a p c", c=512)[c])
                    zp = e3.tile([128, 512], F32, tag="zp", name="zp")
                    v.tensor_copy(zp[:], zpb[:])
                    dma(o_z[c], zp[:])
                    rn = e3.tile([128, 512], F32, tag="rn", name="rn")
                    v.tensor_sub(rn[:], t1[:], zp[:])
                    v.scalar_tensor_tensor(rn[:], rn[:], eta, rp[:],
                                           ALU.mult, ALU.add)
                    dma(o_r[c], rn[:])

    nc.finalize()
    _split_sync_waits(nc, max_waits=1)
    return nc


_CACHE = {}


def _pack_weights(params):
    def bf(x):
        return np.ascontiguousarray(x.astype(_BF))

    out = {}
    for nm, key in (("sfe2", "sfe2_w"), ("d0", "rdb0_dense_w"),
                    ("d1", "rdb1_dense_w"), ("g2", "gff2_w")):
        Wt = np.asarray(params[key], np.float32)  # [64,64,3,3]
        wp = np.zeros((128, 192), np.float32)
        ws = np.zeros((64, 192), np.float32)
        for j in range(3):
            wp[0:64, j * 64 : (j + 1) * 64] = Wt[:, :, 0, j].T
            wp[64:128, j * 64 : (j + 1) * 64] = Wt[:, :, 1, j].T
            ws[0:64, j * 64 : (j + 1) * 64] = Wt[:, :, 2, j].T
        out["w_" + nm + "_p"] = bf(wp)
        out["w_" + nm + "_s"] = bf(ws)
    Wt = np.asarray(params["out_w"], np.float32)  # [3,64,3,3]
    wp = np.zeros((128, 9), np.float32)
    ws = np.zeros((64, 9), np.float32)
    for j in range(3):
        wp[0:64, j * 3 : (j + 1) * 3] = Wt[:, :, 0, j].T
        wp[64:128, j * 3 : (j + 1) * 3] = Wt[:, :, 1, j].T
        ws[0:64, j * 3 : (j + 1) * 3] = Wt[:, :, 2, j].T
    out["w_out_p"] = bf(wp)
    out["w_out_s"] = bf(ws)
    Wt = np.asarray(params["sfe1_w"], np.float32)  # [64,3,3,3]
    w9 = np.zeros((27, 64), np.float32)
    for a in range(3):
        for b in range(3):
            tk = a * 3 + b
            w9[3 * tk : 3 * tk + 3, :] = Wt[:, :, a, b].T
    out["w_sfe1"] = bf(w9)
    for nm, key in (("l0", "rdb0_lff_w"), ("l1", "rdb1_lff_w"),
                    ("g1", "gff1_w")):
        Wt = np.asarray(params[key], np.float32)  # [64,128,1,1]
        out["w_" + nm] = bf(Wt[:, :, 0, 0].T)
    for nm, key in (("sfe1", "sfe1_b"), ("sfe2", "sfe2_b"),
                    ("d0", "rdb0_dense_b"), ("l0", "rdb0_lff_b"),
                    ("d1", "rdb1_dense_b"), ("l1", "rdb1_lff_b"),
                    ("g1", "gff1_b"), ("g2", "gff2_b"), ("out", "out_b")):
        out["b_" + nm] = np.ascontiguousarray(
            np.asarray(params[key], np.float32))
    return out


def kernel(**inputs):
    params = inputs["params"]
    beta = float(np.asarray(params["beta"])[0])
    eta = float(np.asarray(params["eta"])[0])
    lam1 = float(np.asarray(params["lambda_1"])[0])
    lam2 = float(np.asarray(params["lambda_2"])[0])
    wtd = np.asarray(params["t1d_w"], np.float32)[0, :, 0, 0]
    sc = dict(beta=beta, eta=eta, lam1=lam1, lam2=lam2,
              wt0=float(wtd[0]), wt1=float(wtd[1]), wt2=float(wtd[2]))
    key = tuple(sorted(sc.items()))
    if key not in _CACHE:
        _CACHE[key] = _build_program(sc)
    nc = _CACHE[key]

    wpk = _pack_weights(params)

    f32 = lambda x: np.ascontiguousarray(np.asarray(x, np.float32))
    J = f32(inputs["J"])
    means = J.mean(axis=(2, 3), dtype=np.float32)  # [8,3]
    idx = np.argsort(means, axis=1, kind="stable")

    in_maps = []
    for b in range(NCORES):
        P = np.zeros((3, 3), np.float32)
        for k in range(3):
            P[k, idx[b, k]] = 1.0
        psc = np.zeros(16, np.float32)
        psc[0:9] = P.reshape(-1)
        psc[9] = means[b, idx[b, 2]]
        m = {
            "I3": f32(inputs["I"][b]).reshape(3, 128, 512),
            "tp3": f32(inputs["t_p"][b]).reshape(3, 128, 512),
            "bp3": f32(inputs["B_p"][b]).reshape(3, 128, 512),
            "t3": f32(inputs["t"][b]).reshape(3, 128, 512),
            "J3": J[b].reshape(3, 128, 512),
            "Y3": f32(inputs["Y"][b]).reshape(3, 128, 512),
            "Z3": f32(inputs["Z"][b]).reshape(3, 128, 512),
            "Q3": f32(inputs["Q"][b]).reshape(3, 128, 512),
            "R3": f32(inputs["R"][b]).reshape(3, 128, 512),
            "u1": f32(inputs["u"][b]).reshape(128, 512),
            "v1": f32(inputs["v"][b]).reshape(128, 512),
            "w11": f32(inputs["w_1"][b]).reshape(128, 512),
            "w21": f32(inputs["w_2"][b]).reshape(128, 512),
            "psc": psc,
        }
        m.update(wpk)
        in_maps.append(m)

    trace = bool(int(os.environ.get("KERNEL_TRACE", "0")))
    tkw = {}
    if trace:
        try:
            sys.path.insert(0, "/root/problem/work")
            import profhook

            profhook.install()
            tkw = dict(trace=True, tmpdir=os.environ.get(
                "KERNEL_TRACE_DIR", "/root/problem/work/trace_out"))
        except Exception:
            tkw = {}
    r = run_bass_kernel_spmd(nc, in_maps, list(range(NCORES)), **tkw)
    res = r.results
    if trace and getattr(r, "exec_time_ns", None) is not None:
        kernel.last_exec_time_ns = r.exec_time_ns

    B = np.empty((8, 3, H, W), np.float32)
    t_new = np.empty((8, 3, H, W), np.float32)
    Jn = np.empty((8, 3, H, W), np.float32)
    Zn = np.empty((8, 3, H, W), np.float32)
    Qn = np.empty((8, 3, H, W), np.float32)
    Rn = np.empty((8, 3, H, W), np.float32)
    un = np.empty((8, 1, H, W), np.float32)
    vn = np.empty((8, 1, H, W), np.float32)
    w1n = np.empty((8, 1, H, W), np.float32)
    w2n = np.empty((8, 1, H, W), np.float32)
    for b in range(NCORES):
        o = res[b]
        B[b] = o["o_bm"].reshape(3, 1, 1)
        t_new[b] = o["o_t1"].reshape(1, H, W)
        Jn[b] = o["o_j"].reshape(3, H, W)
        Zn[b] = o["o_z"].reshape(3, H, W)
        Qn[b] = o["o_q"].reshape(3, H, W)
        Rn[b] = o["o_r"].reshape(3, H, W)
        un[b, 0] = o["o_u"].reshape(H, W)
        vn[b, 0] = o["o_v"].reshape(H, W)
        w1n[b, 0] = o["o_w1"].reshape(H, W)
        w2n[b, 0] = o["o_w2"].reshape(H, W)
    Y = f32(inputs["Y"])
    return (B, t_new, Jn, Y, Zn, Qn, Rn, un, vn, w1n, w2n,
            f32(params["beta"]))


# revision 29
# speedup vs baseline: 1.8674x; 1.0584x over previous
"""Trainium2 Bass kernel for nn_BasicBlock_1709396984498.

Data-parallel over batch: 1 sample per NeuronCore (8 cores).
Per core: elementwise ADMM phase in fp32 planes [128,512], channel
gather/scatter via host-computed one-hot permutation, RDN conv stack
streamed through padded DRAM maps in bf16 (3x3 convs as 6 matmuls per
512-px tile: 3 row-shift-paired taps at K=128 + 3 singles at K=64).
"""

import json
import os
import sys

for _p in (
    "/root/.axon_site",
    "/root/.axon_site/_ro/trn_rl_repo",
    "/root/.axon_site/_ro/pypackages",
    "/opt/trn_rl_repo",
):
    if os.path.isdir(_p) and _p not in sys.path:
        sys.path.append(_p)

import ml_dtypes
import numpy as np

import concourse.bass as bass
import concourse.bass_isa as bass_isa
import concourse.tile as tile
from concourse import mybir
import concourse.bass_utils as _bu
from concourse.bass_utils import run_bass_kernel_spmd


F32 = mybir.dt.float32
BF16 = mybir.dt.bfloat16
ALU = mybir.AluOpType
ACT = mybir.ActivationFunctionType

GAMMA_1, GAMMA_2 = 0.3, 0.7
H = W = 256
HW = H * W
WP = W + 2  # padded width (258)
PADHW = (H + 2) * WP  # 66564
G = 64
R = 32  # strip rows
NSTRIP = H // R
SLEN = R * WP  # 8256 flat output positions per strip
NCORES = 8

_BF = ml_dtypes.bfloat16


def _split_sync_waits(nc, max_waits=1):
    """Post-pass on the final BIR:
    1) Replace InstLdweights whose weights operand matches the previous
       PE weight load (with only Matmults between) by a NoOp that keeps
       the same sync_info — weight-stationary reuse the toolchain's
       disabled ldw-opt would otherwise do.
    2) Walrus in this env rejects >1 sync-wait on CTRL ops; split
       overflow waits onto preceding same-engine NoOps."""
    import bass_rust

    m = json.loads(bass_rust.module_to_json_string(nc.m))
    for f in m["functions"]:
        for bb in f["blocks"]:
            last_w = None
            for inst in bb["instructions"]:
                op = inst["opcode"]
                if inst.get("engine") != "PE":
                    continue
                if op == "Ldweights":
                    key = json.dumps(inst.get("ins"), sort_keys=True)
                    if key == last_w and not (
                        (inst.get("sync_info") or {}).get("on_update")
                    ):
                        inst["opcode"] = "NoOp"
                        inst["ins"] = []
                        inst["outs"] = []
                    else:
                        last_w = key
                elif op not in ("Matmult",):
                    last_w = None
    n = [0]
    changed = True
    for f in m["functions"]:
        for bb in f["blocks"]:
            out = []
            for inst in bb["instructions"]:
                si = inst.get("sync_info")
                waits = (si or {}).get("on_wait") or []
                if len(waits) > max_waits:
                    changed = True
                    keep = waits[len(waits) - max_waits :]
                    over = waits[: len(waits) - max_waits]
                    for i in range(0, len(over), max_waits):
                        n[0] += 1
                        out.append(
                            {
                                "debug": inst.get("debug", 0),
                                "engine": inst["engine"],
                                "ins": [],
                                "outs": [],
                                "name": f"wsplit-{n[0]}",
                                "opcode": "NoOp",
                                "sync_info": {
                                    "on_update": [],
                                    "on_wait": over[i : i + max_waits],
                                },
                            }
                        )
                    si["on_wait"] = keep
                out.append(inst)
            bb["instructions"] = out
    if changed:
        nc.m = bass_rust.module_from_json_string(json.dumps(m))
    return nc


def _build_program(sc):
    """sc: dict of python-float scalars (beta, eta, lam1, lam2, wt0..2)."""
    beta, eta = sc["beta"], sc["eta"]
    lam1, lam2 = sc["lam1"], sc["lam2"]
    wt0, wt1, wt2 = sc["wt0"], sc["wt1"], sc["wt2"]

    nc = bass.Bass("TRN2", target_bir_lowering=False, debug=False,
                   num_devices=NCORES)

    def din(name, shape, dt=F32):
        return nc.dram_tensor(name, shape, dt, kind="ExternalInput")

    def dout(name, shape, dt=F32):
        return nc.dram_tensor(name, shape, dt, kind="ExternalOutput")

    I3 = din("I3", [3, 128, 512])
    tp3 = din("tp3", [3, 128, 512])
    bp3 = din("bp3", [3, 128, 512])
    t3 = din("t3", [3, 128, 512])
    J3 = din("J3", [3, 128, 512])
    Y3 = din("Y3", [3, 128, 512])
    Z3 = din("Z3", [3, 128, 512])
    Q3 = din("Q3", [3, 128, 512])
    R3 = din("R3", [3, 128, 512])
    u1 = din("u1", [128, 512])
    v1 = din("v1", [128, 512])
    w11 = din("w11", [128, 512])
    w21 = din("w21", [128, 512])
    psc = din("psc", [16])  # P flat (9), jlbar (1), pad

    wq = {}
    for nm in ("sfe2", "d0", "d1", "g2"):
        wq[nm + "_p"] = din("w_" + nm + "_p", [128, 192], BF16)
        wq[nm + "_s"] = din("w_" + nm + "_s", [64, 192], BF16)
    wq["out_p"] = din("w_out_p", [128, 9], BF16)
    wq["out_s"] = din("w_out_s", [64, 9], BF16)
    wq["sfe1"] = din("w_sfe1", [27, 64], BF16)
    for nm in ("l0", "l1", "g1"):
        wq[nm] = din("w_" + nm, [128, 64], BF16)
    bq = {}
    for nm in ("sfe1", "sfe2", "d0", "l0", "d1", "l1", "g1", "g2"):
        bq[nm] = din("b_" + nm, [64])
    bq["out"] = din("b_out", [3])

    o_j = dout("o_j", [3, 128, 512])
    o_q = dout("o_q", [3, 128, 512])
    o_z = dout("o_z", [3, 128, 512])
    o_r = dout("o_r", [3, 128, 512])
    o_t1 = dout("o_t1", [128, 512])
    o_u = dout("o_u", [128, 512])
    o_v = dout("o_v", [128, 512])
    o_w1 = dout("o_w1", [128, 512])
    o_w2 = dout("o_w2", [128, 512])
    o_bm = dout("o_bm", [1, 3])

    with tile.TileContext(nc) as tc:
        from contextlib import ExitStack

        with ExitStack() as ctx:
            persist = ctx.enter_context(tc.tile_pool(name="persist", bufs=1))
            dmaps = ctx.enter_context(
                tc.tile_pool(name="dmaps", bufs=1, space="DRAM"))

            v = nc.vector
            s = nc.scalar
            dma = nc.sync.dma_start

            # ---- persistent tiles ----
            wt = {}
            for nm, h in wq.items():
                wt[nm] = persist.tile(list(h.shape), BF16, tag="w_" + nm, name="w_" + nm)
                dma(wt[nm][:], h[:])
            bt = {}
            for nm, h in bq.items():
                n = h.shape[0]
                bt[nm] = persist.tile([n, 1], F32, tag="b_" + nm, name="b_" + nm)
                dma(bt[nm][:], h[:].rearrange("(p c) -> p c", c=1))

            ones_row = persist.tile([1, 128], F32, tag="ones_row", name="ones_row")
            v.memset(ones_row[:], 1.0)
            ones_col = persist.tile([128, 1], F32, tag="ones_col", name="ones_col")
            v.memset(ones_col[:], 1.0)
            zt = persist.tile([64, 264], BF16, tag="zt", name="zt")
            v.memset(zt[:], 0.0)

            pj_src = persist.tile([1, 16], F32, tag="pj_src", name="pj_src")
            dma(pj_src[0:1, 0:16], psc[:].rearrange("(p c) -> p c", p=1))
            pscal = persist.tile([128, 16], F32, tag="pscal", name="pscal")

            t1 = persist.tile([128, 512], F32, tag="t1", name="t1")
            bscal = persist.tile([128, 4], F32, tag="bscal", name="bscal")
            msrc = persist.tile([1, 4], F32, tag="msrc", name="msrc")
            v.memset(msrc[:], 0.0)

            # ---- DRAM feature maps (bf16, padded 258x258) ----
            maps = {}
            maps["zin"] = dmaps.tile([3, PADHW + 2], BF16, tag="zin", name="zin")
            for nm in ("sfe1", "h0", "d0", "h1", "d1", "h2", "g", "g2"):
                maps[nm] = dmaps.tile([64, PADHW], BF16, tag=nm, name=nm)
            zmap = dmaps.tile([3, HW], BF16, tag="zmap", name="zmap")

            # zero top/bottom pad rows of maps read by 3x3 convs (cheap,
            # contiguous); pad columns are zeroed in SBUF by the producer
            # (store_full), except zin whose planes store is strided.
            for nm in ("sfe1", "h0", "h1", "g", "g2"):
                mp = maps[nm]
                C = mp.shape[0]
                dma(mp[:, 0:WP], zt[0:C, 0:WP])
                dma(mp[:, (WP - 1) * WP : WP * WP], zt[0:C, 0:WP])
            zin_d = maps["zin"][:, 1 : 1 + PADHW]
            dma(zin_d[:, 0:WP], zt[0:3, 0:WP])
            dma(zin_d[:, (WP - 1) * WP : WP * WP], zt[0:3, 0:WP])
            m3 = zin_d.rearrange("p (r c) -> p r c", c=WP)
            dma(m3[:, :, 0:1],
                zt[0:3, 0:WP].rearrange("p (r c) -> p r c", c=1))
            dma(m3[:, :, WP - 1 : WP],
                zt[0:3, 0:WP].rearrange("p (r c) -> p r c", c=1))

            def pA(k):
                return pscal[:, k : k + 1]

            # ================= E1 + E2 (elementwise) =================
            with tc.tile_pool(name="ew", bufs=1) as ew, \
                 tc.tile_pool(name="psB", bufs=1, space="PSUM") as psB:
                def pl(name):
                    return ew.tile([128, 512], F32, tag="pl_" + name, name="pl_" + name)

                ps_p = psB.tile([128, 16], F32, tag="t_bc", name="t_bc")
                nc.tensor.matmul(ps_p[:], ones_row[:], pj_src[:],
                                 start=True, stop=True)
                s.copy(pscal[:], ps_p[:])

                pj = [pl(f"J{c}") for c in range(3)]
                for c in range(3):
                    dma(pj[c][:], J3[c])
                pu, pv = pl("u"), pl("v")
                pw1, pw2 = pl("w1"), pl("w2")
                dma(pu[:], u1[:]); dma(pv[:], v1[:])
                dma(pw1[:], w11[:]); dma(pw2[:], w21[:])

                js = [pl(f"js{k}") for k in range(3)]
                for k in range(3):
                    v.tensor_scalar_mul(js[k][:], pj[0][:], pA(3 * k + 0))
                    v.scalar_tensor_tensor(js[k][:], pj[1][:], pA(3 * k + 1),
                                           js[k][:], ALU.mult, ALU.add)
                    v.scalar_tensor_tensor(js[k][:], pj[2][:], pA(3 * k + 2),
                                           js[k][:], ALU.mult, ALU.add)
                jl = js[2]

                x1, x2 = pl("x1"), pl("x2")
                v.scalar_tensor_tensor(x1[:], jl[:], -1.0, pu[:],
                                       ALU.mult, ALU.add)
                v.tensor_scalar_add(x1[:], x1[:], pA(9))
                v.scalar_tensor_tensor(x2[:], jl[:], -1.0, pv[:],
                                       ALU.mult, ALU.add)
                v.tensor_scalar_add(x2[:], x2[:], pA(9))

                tmp, tmp2 = pl("tmp"), pl("tmp2")
                # u_new / v_new (soft threshold)
                v.tensor_scalar(tmp[:], x1[:], 1.0 / lam1, -1.0 / lam1,
                                ALU.min, ALU.max)
                v.tensor_sub(tmp[:], x1[:], tmp[:])
                dma(o_u[:], tmp[:])
                v.tensor_scalar(tmp[:], x2[:], 1.0 / lam2, -1.0 / lam2,
                                ALU.min, ALU.max)
                v.tensor_sub(tmp[:], x2[:], tmp[:])
                dma(o_v[:], tmp[:])
                # w_new (clip)
                v.tensor_scalar(tmp[:], x1[:], lam1, 1.0, ALU.mult, ALU.min)
                v.tensor_scalar_max(tmp[:], tmp[:], -1.0)
                dma(o_w1[:], tmp[:])
                v.tensor_scalar(tmp[:], x2[:], lam2, 1.0, ALU.mult, ALU.min)
                v.tensor_scalar_max(tmp[:], tmp[:], -1.0)
                dma(o_w2[:], tmp[:])

                # new sorted-channel values
                jm_n, js_n = pl("jm_n"), pl("js_n")
                v.scalar_tensor_tensor(tmp[:], pw1[:], -1.0 / lam1, x1[:],
                                       ALU.mult, ALU.add)
                v.tensor_mul(tmp[:], tmp[:], jl[:])
                v.tensor_add(jm_n[:], tmp[:], js[1][:])
                v.scalar_tensor_tensor(tmp[:], pw2[:], -1.0 / lam2, x2[:],
                                       ALU.mult, ALU.add)
                v.tensor_mul(tmp[:], tmp[:], jl[:])
                v.tensor_add(js_n[:], tmp[:], js[0][:])

                comp = [js_n, jm_n, jl]
                jsc = [pl(f"jsc{c}") for c in range(3)]
                for c in range(3):
                    v.tensor_scalar_mul(jsc[c][:], comp[0][:], pA(0 + c))
                    v.scalar_tensor_tensor(jsc[c][:], comp[1][:], pA(3 + c),
                                           jsc[c][:], ALU.mult, ALU.add)
                    v.scalar_tensor_tensor(jsc[c][:], comp[2][:], pA(6 + c),
                                           jsc[c][:], ALU.mult, ALU.add)

                # ---- E2 ----
                pi = [pl(f"I{c}") for c in range(3)]
                pt = [pl(f"t{c}") for c in range(3)]
                for c in range(3):
                    dma(pi[c][:], I3[c]); dma(pt[c][:], t3[c])
                for c in range(3):
                    pbp = pl(f"bp{c}")
                    dma(pbp[:], bp3[c])
                    ta = pl(f"e2a{c}")
                    tb = pl(f"e2b{c}")
                    omt = pl(f"omt{c}")
                    v.tensor_scalar(omt[:], pt[c][:], -1.0, 1.0,
                                    ALU.mult, ALU.add)
                    v.tensor_mul(ta[:], jsc[c][:], pt[c][:])
                    v.tensor_sub(ta[:], ta[:], pi[c][:])
                    v.tensor_mul(ta[:], ta[:], omt[:])
                    v.scalar_tensor_tensor(ta[:], pbp[:], GAMMA_1, ta[:],
                                           ALU.mult, ALU.subtract)
                    v.tensor_mul(tb[:], omt[:], omt[:])
                    v.tensor_scalar_add(tb[:], tb[:], GAMMA_1)
                    rcp = pl(f"rcp{c}")
                    v.reciprocal(rcp[:], tb[:])
                    v.tensor_mul(ta[:], ta[:], rcp[:])
                    prt = pl(f"prt{c}")
                    v.tensor_reduce(prt[:, 0:1], ta[:],
                                    mybir.AxisListType.X, ALU.add)
                    ps_m = psB.tile([1, 1], F32, tag="t_m", name="t_m")
                    nc.tensor.matmul(ps_m[:], prt[:, 0:1], ones_col[:],
                                     start=True, stop=True)
                    s.mul(msrc[0:1, c : c + 1], ps_m[:], 1.0 / HW)
                ps_b = psB.tile([128, 4], F32, tag="t_bc2", name="t_bc2")
                nc.tensor.matmul(ps_b[:], ones_row[:], msrc[:],
                                 start=True, stop=True)
                s.copy(bscal[:], ps_b[:])
                dma(o_bm[:], msrc[0:1, 0:3])

                tt = [pl(f"tt{c}") for c in range(3)]
                prcs = [pl(f"Rc{c}") for c in range(3)]
                for c in range(3):
                    pzc = pl(f"Zc{c}")
                    dma(pzc[:], Z3[c]); dma(prcs[c][:], R3[c])
                    ta = pl(f"e2c{c}")
                    tb = pl(f"e2d{c}")
                    jb = pl(f"jb{c}")
                    v.tensor_scalar_sub(jb[:], jsc[c][:],
                                        bscal[:, c : c + 1])
                    v.tensor_scalar(ta[:], pi[c][:], -1.0,
                                    bscal[:, c : c + 1], ALU.mult, ALU.add)
                    v.tensor_mul(ta[:], ta[:], jb[:])
                    v.scalar_tensor_tensor(tb[:], pzc[:], eta, prcs[c][:],
                                           ALU.mult, ALU.subtract)
                    ptp = pl(f"tpc{c}")
                    dma(ptp[:], tp3[c])
                    v.scalar_tensor_tensor(tb[:], ptp[:], GAMMA_2, tb[:],
                                           ALU.mult, ALU.add)
                    v.tensor_sub(tb[:], tb[:], ta[:])
                    v.tensor_mul(ta[:], jb[:], jb[:])
                    v.tensor_scalar_add(ta[:], ta[:], GAMMA_2 + eta)
                    rcp2 = pl(f"rcp2{c}")
                    v.reciprocal(rcp2[:], ta[:])
                    v.tensor_mul(tt[c][:], tb[:], rcp2[:])

                v.tensor_scalar_mul(tmp[:], tt[0][:], wt0)
                v.scalar_tensor_tensor(tmp[:], tt[1][:], wt1, tmp[:],
                                       ALU.mult, ALU.add)
                v.scalar_tensor_tensor(tmp[:], tt[2][:], wt2, tmp[:],
                                       ALU.mult, ALU.add)
                s.activation(t1[:], tmp[:], ACT.Relu)
                dma(o_t1[:], t1[:])

                rdj = pl("rdj")
                v.tensor_mul(tmp[:], t1[:], t1[:])
                v.tensor_scalar_add(tmp[:], tmp[:], beta)
                v.reciprocal(rdj[:], tmp[:])
                omt1 = pl("omt1")
                v.tensor_scalar(omt1[:], t1[:], -1.0, 1.0, ALU.mult, ALU.add)

                for c in range(3):
                    py = pl(f"Yc{c}")
                    pq = pl(f"Qc{c}")
                    dma(py[:], Y3[c]); dma(pq[:], Q3[c])
                    ta = pl(f"e2e{c}")
                    tb = pl(f"e2f{c}")
                    v.tensor_scalar_mul(ta[:], omt1[:], bscal[:, c : c + 1])
                    v.tensor_sub(ta[:], ta[:], pi[c][:])
                    v.tensor_mul(ta[:], ta[:], t1[:])
                    v.scalar_tensor_tensor(tb[:], py[:], beta, pq[:],
                                           ALU.mult, ALU.subtract)
                    v.tensor_sub(tb[:], tb[:], ta[:])
                    jn = pl(f"jn{c}")
                    v.tensor_mul(jn[:], tb[:], rdj[:])
                    dma(o_j[c], jn[:])
                    v.tensor_sub(ta[:], jn[:], py[:])
                    v.scalar_tensor_tensor(ta[:], ta[:], beta, pq[:],
                                           ALU.mult, ALU.add)
                    dma(o_q[c], ta[:])
                    # Zin
                    v.scalar_tensor_tensor(tb[:], prcs[c][:], 1.0 / eta,
                                           t1[:], ALU.mult, ALU.add)
                    zb = ew.tile([128, 512], BF16, tag=f"pl_zinbf{c}",
                                 name=f"pl_zinbf{c}")
                    zb_b = zb[:]
                    v.tensor_copy(zb_b, tb[:])
                    dst = maps["zin"][c : c + 1, 1 : 1 + PADHW].rearrange(
                        "a (r c) -> a r c", c=WP)[0:1, 1:257, 1:257]
                    dst = dst.rearrange("a (p r) c -> (a p) r c", p=128)
                    src = zb_b.rearrange("p (r c) -> p r c", c=256)
                    dma(dst, src)

            # ================= RDN conv stack =================
            CIN_COLS = (R + 2) * WP + 12  # 8784
            with tc.tile_pool(name="cin", bufs=2) as cin, \
                 tc.tile_pool(name="cout", bufs=2) as cout, \
                 tc.tile_pool(name="cres", bufs=2) as cres, \
                 tc.tile_pool(name="psA", bufs=8, space="PSUM") as psA:

                def tiles_of_strip():
                    q0 = 0
                    out = []
                    while q0 < SLEN:
                        out.append((q0, min(512, SLEN - q0)))
                        q0 += 512
                    return out

                def store_interior(src_buf, dst_map, y0, Cn):
                    s3 = src_buf[:].rearrange("p (r c) -> p r c", c=WP)
                    v.memset(s3[:, :, 0:1], 0.0)
                    v.memset(s3[:, :, WP - 1 : WP], 0.0)
                    dma(dst_map[:, (y0 + 1) * WP : (y0 + 1 + R) * WP],
                        src_buf[:])

                def conv3x3(src_map, dst_map, wp_t, ws_t, bias_t, relu,
                            resid_map=None):
                    M = 64
                    for st in range(NSTRIP):
                        y0 = st * R
                        it = cin.tile([128, CIN_COLS], BF16, tag="cin", name="cin")
                        dma(it[0:64, 1 : 1 + (R + 2) * WP],
                            src_map[:, y0 * WP : (y0 + R + 2) * WP])
                        dma(it[64:128, 1 : 1 + (R + 1) * WP],
                            it[0:64, 1 + WP : 1 + (R + 2) * WP])
                        ob = cout.tile([64, SLEN], BF16, tag="cout", name="cout")
                        if resid_map is not None:
                            rs = cres.tile([64, SLEN], BF16, tag="cres", name="cres")
                            dma(rs[:], resid_map[:,
                                (y0 + 1) * WP : (y0 + 1) * WP + SLEN])
                        tl = tiles_of_strip()
                        for g0 in range(0, len(tl), 4):
                            grp = tl[g0 : g0 + 4]
                            pss = [psA.tile([64, n], F32, tag="ps",
                                            name="ps") for (_, n) in grp]
                            for j in range(3):
                                for (q0, n), ps in zip(grp, pss):
                                    nc.tensor.matmul(
                                        ps[:], wp_t[:, j * M : (j + 1) * M],
                                        it[0:128, q0 + j : q0 + j + n],
                                        start=(j == 0), stop=False)
                            for j in range(3):
                                for (q0, n), ps in zip(grp, pss):
                                    nc.tensor.matmul(
                                        ps[:],
                                        ws_t[0:64, j * M : (j + 1) * M],
                                        it[0:64,
                                           q0 + 516 + j : q0 + 516 + j + n],
                                        start=False, stop=(j == 2))
                            for gi, ((q0, n), ps) in enumerate(
                                    zip(grp, pss)):
                                obs = ob[:, q0 : q0 + n]
                                if gi % 2 == 0:
                                    s.activation(
                                        obs, ps[:],
                                        ACT.Relu if relu else ACT.Identity,
                                        bias=bias_t[:])
                                elif relu:
                                    v.tensor_scalar(obs, ps[:], bias_t[:],
                                                    0.0, ALU.add, ALU.max)
                                else:
                                    v.tensor_scalar_add(obs, ps[:],
                                                        bias_t[:])
                                if resid_map is not None:
                                    v.tensor_add(obs, obs,
                                                 rs[:, q0 : q0 + n])
                        store_interior(ob, dst_map, y0, 64)

                def conv1x1(a_map, b_map, dst_map, w_t, bias_t, resid):
                    for st in range(NSTRIP):
                        y0 = st * R
                        it = cin.tile([128, CIN_COLS], BF16, tag="cin", name="cin")
                        dma(it[0:64, 0:SLEN],
                            a_map[:, (y0 + 1) * WP : (y0 + 1) * WP + SLEN])
                        dma(it[64:128, 0:SLEN],
                            b_map[:, (y0 + 1) * WP : (y0 + 1) * WP + SLEN])
                        ob = cout.tile([64, SLEN], BF16, tag="cout", name="cout")
                        for ti, (q0, n) in enumerate(tiles_of_strip()):
                            ps = psA.tile([64, n], F32, tag="ps", name="ps")
                            nc.tensor.matmul(ps[:], w_t[:],
                                             it[0:128, q0 : q0 + n],
                                             start=True, stop=True)
                            obs = ob[:, q0 : q0 + n]
                            if ti % 2 == 0:
                                s.activation(obs, ps[:], ACT.Identity,
                                             bias=bias_t[:])
                            else:
                                v.tensor_scalar_add(obs, ps[:], bias_t[:])
                            if resid:
                                v.tensor_add(obs, obs, it[0:64, q0 : q0 + n])
                        store_interior(ob, dst_map, y0, 64)

                def conv_sfe1(src_map, dst_map, w_t, bias_t):
                    D = 2
                    for st in range(NSTRIP):
                        y0 = st * R
                        it = cin.tile([27, CIN_COLS], BF16, tag="cin9", name="cin9")
                        for a in range(3):
                            src3 = bass.AP(
                                src_map.tensor, y0 * WP + a * WP,
                                [[1, 3], [PADHW + 2, 3], [1, SLEN]])
                            dma(it[9 * a : 9 * a + 9, D : D + SLEN], src3)
                        ob = cout.tile([64, SLEN], BF16, tag="cout", name="cout")
                        for (q0, n) in tiles_of_strip():
                            ps = psA.tile([64, n], F32, tag="ps", name="ps")
                            nc.tensor.matmul(ps[:], w_t[:],
                                             it[0:27, D + q0 : D + q0 + n],
                                             start=True, stop=True)
                            s.activation(ob[:, q0 : q0 + n], ps[:],
                                         ACT.Identity, bias=bias_t[:])
                        store_interior(ob, dst_map, y0, 64)

                def conv_out(src_map, wp_t, ws_t, bias_t):
                    for st in range(NSTRIP):
                        y0 = st * R
                        it = cin.tile([128, CIN_COLS], BF16, tag="cin", name="cin")
                        dma(it[0:64, 1 : 1 + (R + 2) * WP],
                            src_map[:, y0 * WP : (y0 + R + 2) * WP])
                        dma(it[64:128, 1 : 1 + (R + 1) * WP],
                            it[0:64, 1 + WP : 1 + (R + 2) * WP])
                        zb = cres.tile([3, SLEN], BF16, tag="zbuf", name="zbuf", bufs=1)
                        tl = tiles_of_strip()
                        for g0 in range(0, len(tl), 4):
                            grp = tl[g0 : g0 + 4]
                            pss = [psA.tile([3, n], F32, tag="ps",
                                            name="ps") for (_, n) in grp]
                            for j in range(3):
                                for (q0, n), ps in zip(grp, pss):
                                    nc.tensor.matmul(
                                        ps[:], wp_t[:, j * 3 : (j + 1) * 3],
                                        it[0:128, q0 + j : q0 + j + n],
                                        start=(j == 0), stop=False)
                            for j in range(3):
                                for (q0, n), ps in zip(grp, pss):
                                    nc.tensor.matmul(
                                        ps[:],
                                        ws_t[0:64, j * 3 : (j + 1) * 3],
                                        it[0:64,
                                           q0 + 516 + j : q0 + 516 + j + n],
                                        start=False, stop=(j == 2))
                            for (q0, n), ps in zip(grp, pss):
                                s.activation(zb[:, q0 : q0 + n], ps[:],
                                             ACT.Identity, bias=bias_t[:])
                        src = zb[:].rearrange("p (r c) -> p r c", c=WP)
                        src = src[:, :, 1:257]
                        dst = zmap[:].rearrange("p (r c) -> p r c", c=256)
                        dst = dst[:, y0 : y0 + R, :]
                        dma(dst, src)

                conv_sfe1(maps["zin"], maps["sfe1"], wt["sfe1"], bt["sfe1"])
                conv3x3(maps["sfe1"], maps["h0"], wt["sfe2_p"], wt["sfe2_s"],
                        bt["sfe2"], relu=False)
                conv3x3(maps["h0"], maps["d0"], wt["d0_p"], wt["d0_s"],
                        bt["d0"], relu=True)
                conv1x1(maps["h0"], maps["d0"], maps["h1"], wt["l0"],
                        bt["l0"], resid=True)
                conv3x3(maps["h1"], maps["d1"], wt["d1_p"], wt["d1_s"],
                        bt["d1"], relu=True)
                conv1x1(maps["h1"], maps["d1"], maps["h2"], wt["l1"],
                        bt["l1"], resid=True)
                conv1x1(maps["h1"], maps["h2"], maps["g"], wt["g1"],
                        bt["g1"], resid=False)
                conv3x3(maps["g"], maps["g2"], wt["g2_p"], wt["g2_s"],
                        bt["g2"], relu=False, resid_map=maps["sfe1"])
                conv_out(maps["g2"], wt["out_p"], wt["out_s"], bt["out"])

            # ================= E3 =================
            with tc.tile_pool(name="e3", bufs=2) as e3:
                for c in range(3):
                    rp = e3.tile([128, 512], F32, tag="rp", name="rp")
                    dma(rp[:], R3[c])
                    zpb = e3.tile([128, 512], BF16, tag="zpb", name="zpb")
                    dma(zpb[:], zmap[:].rearrange(
                        "a (p c) -> a p c", c=512)[c])
                    zp = e3.tile([128, 512], F32, tag="zp", name="zp")
                    v.tensor_copy(zp[:], zpb[:])
                    dma(o_z[c], zp[:])
                    rn = e3.tile([128, 512], F32, tag="rn", name="rn")
                    v.tensor_sub(rn[:], t1[:], zp[:])
                    v.scalar_tensor_tensor(rn[:], rn[:], eta, rp[:],
                                           ALU.mult, ALU.add)
                    dma(o_r[c], rn[:])

    nc.finalize()
    _split_sync_waits(nc, max_waits=1)
    return nc


_CACHE = {}


def _pack_weights(params):
    def bf(x):
        return np.ascontiguousarray(x.astype(_BF))

    out = {}
    for nm, key in (("sfe2", "sfe2_w"), ("d0", "rdb0_dense_w"),
                    ("d1", "rdb1_dense_w"), ("g2", "gff2_w")):
        Wt = np.asarray(params[key], np.float32)  # [64,64,3,3]
        wp = np.zeros((128, 192), np.float32)
        ws = np.zeros((64, 192), np.float32)
        for j in range(3):
            wp[0:64, j * 64 : (j + 1) * 64] = Wt[:, :, 0, j].T
            wp[64:128, j * 64 : (j + 1) * 64] = Wt[:, :, 1, j].T
            ws[0:64, j * 64 : (j + 1) * 64] = Wt[:, :, 2, j].T
        out["w_" + nm + "_p"] = bf(wp)
        out["w_" + nm + "_s"] = bf(ws)
    Wt = np.asarray(params["out_w"], np.float32)  # [3,64,3,3]
    wp = np.zeros((128, 9), np.float32)
    ws = np.zeros((64, 9), np.float32)
    for j in range(3):
        wp[0:64, j * 3 : (j + 1) * 3] = Wt[:, :, 0, j].T
        wp[64:128, j * 3 : (j + 1) * 3] = Wt[:, :, 1, j].T
        ws[0:64, j * 3 : (j + 1) * 3] = Wt[:, :, 2, j].T
    out["w_out_p"] = bf(wp)
    out["w_out_s"] = bf(ws)
    Wt = np.asarray(params["sfe1_w"], np.float32)  # [64,3,3,3]
    w9 = np.zeros((27, 64), np.float32)
    for a in range(3):
        for b in range(3):
            tk = a * 3 + b
            w9[3 * tk : 3 * tk + 3, :] = Wt[:, :, a, b].T
    out["w_sfe1"] = bf(w9)
    for nm, key in (("l0", "rdb0_lff_w"), ("l1", "rdb1_lff_w"),
                    ("g1", "gff1_w")):
        Wt = np.asarray(params[key], np.float32)  # [64,128,1,1]
        out["w_" + nm] = bf(Wt[:, :, 0, 0].T)
    for nm, key in (("sfe1", "sfe1_b"), ("sfe2", "sfe2_b"),
                    ("d0", "rdb0_dense_b"), ("l0", "rdb0_lff_b"),
                    ("d1", "rdb1_dense_b"), ("l1", "rdb1_lff_b"),
                    ("g1", "gff1_b"), ("g2", "gff2_b"), ("out", "out_b")):
        out["b_" + nm] = np.ascontiguousarray(
            np.asarray(params[key], np.float32))
    return out


def kernel(**inputs):
    params = inputs["params"]
    beta = float(np.asarray(params["beta"])[0])
    eta = float(np.asarray(params["eta"])[0])
    lam1 = float(np.asarray(params["lambda_1"])[0])
    lam2 = float(np.asarray(params["lambda_2"])[0])
    wtd = np.asarray(params["t1d_w"], np.float32)[0, :, 0, 0]
    sc = dict(beta=beta, eta=eta, lam1=lam1, lam2=lam2,
              wt0=float(wtd[0]), wt1=float(wtd[1]), wt2=float(wtd[2]))
    key = tuple(sorted(sc.items()))
    if key not in _CACHE:
        _CACHE[key] = _build_program(sc)
    nc = _CACHE[key]

    wpk = _pack_weights(params)

    f32 = lambda x: np.ascontiguousarray(np.asarray(x, np.float32))
    J = f32(inputs["J"])
    means = J.mean(axis=(2, 3), dtype=np.float32)  # [8,3]
    idx = np.argsort(means, axis=1, kind="stable")

    in_maps = []
    for b in range(NCORES):
        P = np.zeros((3, 3), np.float32)
        for k in range(3):
            P[k, idx[b, k]] = 1.0
        psc = np.zeros(16, np.float32)
        psc[0:9] = P.reshape(-1)
        psc[9] = means[b, idx[b, 2]]
        m = {
            "I3": f32(inputs["I"][b]).reshape(3, 128, 512),
            "tp3": f32(inputs["t_p"][b]).reshape(3, 128, 512),
            "bp3": f32(inputs["B_p"][b]).reshape(3, 128, 512),
            "t3": f32(inputs["t"][b]).reshape(3, 128, 512),
            "J3": J[b].reshape(3, 128, 512),
            "Y3": f32(inputs["Y"][b]).reshape(3, 128, 512),
            "Z3": f32(inputs["Z"][b]).reshape(3, 128, 512),
            "Q3": f32(inputs["Q"][b]).reshape(3, 128, 512),
            "R3": f32(inputs["R"][b]).reshape(3, 128, 512),
            "u1": f32(inputs["u"][b]).reshape(128, 512),
            "v1": f32(inputs["v"][b]).reshape(128, 512),
            "w11": f32(inputs["w_1"][b]).reshape(128, 512),
            "w21": f32(inputs["w_2"][b]).reshape(128, 512),
            "psc": psc,
        }
        m.update(wpk)
        in_maps.append(m)

    trace = bool(int(os.environ.get("KERNEL_TRACE", "0")))
    tkw = {}
    if trace:
        try:
            sys.path.insert(0, "/root/problem/work")
            import profhook

            profhook.install()
            tkw = dict(trace=True, tmpdir=os.environ.get(
                "KERNEL_TRACE_DIR", "/root/problem/work/trace_out"))
        except Exception:
            tkw = {}
    r = run_bass_kernel_spmd(nc, in_maps, list(range(NCORES)), **tkw)
    res = r.results
    if trace and getattr(r, "exec_time_ns", None) is not None:
        kernel.last_exec_time_ns = r.exec_time_ns

    B = np.empty((8, 3, H, W), np.float32)
    t_new = np.empty((8, 3, H, W), np.float32)
    Jn = np.empty((8, 3, H, W), np.float32)
    Zn = np.empty((8, 3, H, W), np.float32)
    Qn = np.empty((8, 3, H, W), np.float32)
    Rn = np.empty((8, 3, H, W), np.float32)
    un = np.empty((8, 1, H, W), np.float32)
    vn = np.empty((8, 1, H, W), np.float32)
    w1n = np.empty((8, 1, H, W), np.float32)
    w2n = np.empty((8, 1, H, W), np.float32)
    for b in range(NCORES):
        o = res[b]
        B[b] = o["o_bm"].reshape(3, 1, 1)
        t_new[b] = o["o_t1"].reshape(1, H, W)
        Jn[b] = o["o_j"].reshape(3, H, W)
        Zn[b] = o["o_z"].reshape(3, H, W)
        Qn[b] = o["o_q"].reshape(3, H, W)
        Rn[b] = o["o_r"].reshape(3, H, W)
        un[b, 0] = o["o_u"].reshape(H, W)
        vn[b, 0] = o["o_v"].reshape(H, W)
        w1n[b, 0] = o["o_w1"].reshape(H, W)
        w2n[b, 0] = o["o_w2"].reshape(H, W)
    Y = f32(inputs["Y"])
    return (B, t_new, Jn, Y, Zn, Qn, Rn, un, vn, w1n, w2n,
            f32(params["beta"]))


# revision 30
# speedup vs baseline: 1.9721x; 1.0561x over previous
"""Trainium2 Bass kernel for nn_BasicBlock_1709396984498.

Data-parallel over batch: 1 sample per NeuronCore (8 cores).
Per core: elementwise ADMM phase in fp32 planes [128,512], channel
gather/scatter via host-computed one-hot permutation, RDN conv stack
streamed through padded DRAM maps in bf16 (3x3 convs as 6 matmuls per
512-px tile: 3 row-shift-paired taps at K=128 + 3 singles at K=64).
"""

import json
import os
import sys

for _p in (
    "/root/.axon_site",
    "/root/.axon_site/_ro/trn_rl_repo",
    "/root/.axon_site/_ro/pypackages",
    "/opt/trn_rl_repo",
):
    if os.path.isdir(_p) and _p not in sys.path:
        sys.path.append(_p)

import ml_dtypes
import numpy as np

import concourse.bass as bass
import concourse.bass_isa as bass_isa
import concourse.tile as tile
from concourse import mybir
import concourse.bass_utils as _bu
from concourse.bass_utils import run_bass_kernel_spmd


F32 = mybir.dt.float32
BF16 = mybir.dt.bfloat16
ALU = mybir.AluOpType
ACT = mybir.ActivationFunctionType

GAMMA_1, GAMMA_2 = 0.3, 0.7
H = W = 256
HW = H * W
WP = W + 2  # padded width (258)
PADHW = (H + 2) * WP  # 66564
G = 64
R = 32  # strip rows
NSTRIP = H // R
SLEN = R * WP  # 8256 flat output positions per strip
NCORES = 8

_BF = ml_dtypes.bfloat16


def _split_sync_waits(nc, max_waits=1):
    """Post-pass on the final BIR:
    1) Replace InstLdweights whose weights operand matches the previous
       PE weight load (with only Matmults between) by a NoOp that keeps
       the same sync_info — weight-stationary reuse the toolchain's
       disabled ldw-opt would otherwise do.
    2) Walrus in this env rejects >1 sync-wait on CTRL ops; split
       overflow waits onto preceding same-engine NoOps."""
    import bass_rust

    m = json.loads(bass_rust.module_to_json_string(nc.m))
    for f in m["functions"]:
        for bb in f["blocks"]:
            last_w = None
            for inst in bb["instructions"]:
                op = inst["opcode"]
                if inst.get("engine") != "PE":
                    continue
                if op == "Ldweights":
                    key = json.dumps(inst.get("ins"), sort_keys=True)
                    if key == last_w and not (
                        (inst.get("sync_info") or {}).get("on_update")
                    ):
                        inst["opcode"] = "NoOp"
                        inst["ins"] = []
                        inst["outs"] = []
                    else:
                        last_w = key
                elif op not in ("Matmult",):
                    last_w = None
    n = [0]
    changed = True
    for f in m["functions"]:
        for bb in f["blocks"]:
            out = []
            for inst in bb["instructions"]:
                si = inst.get("sync_info")
                waits = (si or {}).get("on_wait") or []
                if len(waits) > max_waits:
                    changed = True
                    keep = waits[len(waits) - max_waits :]
                    over = waits[: len(waits) - max_waits]
                    for i in range(0, len(over), max_waits):
                        n[0] += 1
                        out.append(
                            {
                                "debug": inst.get("debug", 0),
                                "engine": inst["engine"],
                                "ins": [],
                                "outs": [],
                                "name": f"wsplit-{n[0]}",
                                "opcode": "NoOp",
                                "sync_info": {
                                    "on_update": [],
                                    "on_wait": over[i : i + max_waits],
                                },
                            }
                        )
                    si["on_wait"] = keep
                out.append(inst)
            bb["instructions"] = out
    if changed:
        nc.m = bass_rust.module_from_json_string(json.dumps(m))
    return nc


def _build_program(sc):
    """sc: dict of python-float scalars (beta, eta, lam1, lam2, wt0..2)."""
    beta, eta = sc["beta"], sc["eta"]
    lam1, lam2 = sc["lam1"], sc["lam2"]
    wt0, wt1, wt2 = sc["wt0"], sc["wt1"], sc["wt2"]

    nc = bass.Bass("TRN2", target_bir_lowering=False, debug=False,
                   num_devices=NCORES)

    def din(name, shape, dt=F32):
        return nc.dram_tensor(name, shape, dt, kind="ExternalInput")

    def dout(name, shape, dt=F32):
        return nc.dram_tensor(name, shape, dt, kind="ExternalOutput")

    I3 = din("I3", [3, 128, 512])
    tp3 = din("tp3", [3, 128, 512])
    bp3 = din("bp3", [3, 128, 512])
    t3 = din("t3", [3, 128, 512])
    J3 = din("J3", [3, 128, 512])
    Y3 = din("Y3", [3, 128, 512])
    Z3 = din("Z3", [3, 128, 512])
    Q3 = din("Q3", [3, 128, 512])
    R3 = din("R3", [3, 128, 512])
    u1 = din("u1", [128, 512])
    v1 = din("v1", [128, 512])
    w11 = din("w11", [128, 512])
    w21 = din("w21", [128, 512])
    psc = din("psc", [16])  # P flat (9), jlbar (1), pad

    wq = {}
    for nm in ("sfe2", "d0", "d1", "g2"):
        wq[nm + "_p"] = din("w_" + nm + "_p", [128, 192], BF16)
        wq[nm + "_s"] = din("w_" + nm + "_s", [64, 192], BF16)
    wq["out_p"] = din("w_out_p", [128, 9], BF16)
    wq["out_s"] = din("w_out_s", [64, 9], BF16)
    wq["sfe1"] = din("w_sfe1", [27, 64], BF16)
    for nm in ("l0", "l1", "g1"):
        wq[nm] = din("w_" + nm, [128, 64], BF16)
    bq = {}
    for nm in ("sfe1", "sfe2", "d0", "l0", "d1", "l1", "g1", "g2"):
        bq[nm] = din("b_" + nm, [64])
    bq["out"] = din("b_out", [3])

    o_j = dout("o_j", [3, 128, 512])
    o_q = dout("o_q", [3, 128, 512])
    o_z = dout("o_z", [3, 128, 512])
    o_r = dout("o_r", [3, 128, 512])
    o_t1 = dout("o_t1", [128, 512])
    o_u = dout("o_u", [128, 512])
    o_v = dout("o_v", [128, 512])
    o_w1 = dout("o_w1", [128, 512])
    o_w2 = dout("o_w2", [128, 512])
    o_bm = dout("o_bm", [1, 3])

    with tile.TileContext(nc) as tc:
        from contextlib import ExitStack

        with ExitStack() as ctx:
            persist = ctx.enter_context(tc.tile_pool(name="persist", bufs=1))
            dmaps = ctx.enter_context(
                tc.tile_pool(name="dmaps", bufs=1, space="DRAM"))

            v = nc.vector
            s = nc.scalar
            dma = nc.sync.dma_start

            # ---- persistent tiles ----
            wt = {}
            for nm, h in wq.items():
                wt[nm] = persist.tile(list(h.shape), BF16, tag="w_" + nm, name="w_" + nm)
                dma(wt[nm][:], h[:])
            bt = {}
            for nm, h in bq.items():
                n = h.shape[0]
                bt[nm] = persist.tile([n, 1], F32, tag="b_" + nm, name="b_" + nm)
                dma(bt[nm][:], h[:].rearrange("(p c) -> p c", c=1))

            ones_row = persist.tile([1, 128], F32, tag="ones_row", name="ones_row")
            v.memset(ones_row[:], 1.0)
            ones_col = persist.tile([128, 1], F32, tag="ones_col", name="ones_col")
            v.memset(ones_col[:], 1.0)
            zt = persist.tile([64, 264], BF16, tag="zt", name="zt")
            v.memset(zt[:], 0.0)

            pj_src = persist.tile([1, 16], F32, tag="pj_src", name="pj_src")
            dma(pj_src[0:1, 0:16], psc[:].rearrange("(p c) -> p c", p=1))
            pscal = persist.tile([128, 16], F32, tag="pscal", name="pscal")

            t1 = persist.tile([128, 512], F32, tag="t1", name="t1")
            bscal = persist.tile([128, 4], F32, tag="bscal", name="bscal")
            msrc = persist.tile([1, 4], F32, tag="msrc", name="msrc")
            v.memset(msrc[:], 0.0)

            # ---- DRAM feature maps (bf16, padded 258x258) ----
            maps = {}
            maps["zin"] = dmaps.tile([3, PADHW + 2], BF16, tag="zin", name="zin")
            for nm in ("sfe1", "h0", "d0", "h1", "d1", "h2", "g", "g2"):
                maps[nm] = dmaps.tile([64, PADHW], BF16, tag=nm, name=nm)
            zmap = dmaps.tile([3, HW], BF16, tag="zmap", name="zmap")

            # zero top/bottom pad rows of maps read by 3x3 convs (cheap,
            # contiguous); pad columns are zeroed in SBUF by the producer
            # (store_full), except zin whose planes store is strided.
            for nm in ("sfe1", "h0", "h1", "g", "g2"):
                mp = maps[nm]
                C = mp.shape[0]
                dma(mp[:, 0:WP], zt[0:C, 0:WP])
                dma(mp[:, (WP - 1) * WP : WP * WP], zt[0:C, 0:WP])
            zin_d = maps["zin"][:, 1 : 1 + PADHW]
            dma(zin_d[:, 0:WP], zt[0:3, 0:WP])
            dma(zin_d[:, (WP - 1) * WP : WP * WP], zt[0:3, 0:WP])
            m3 = zin_d.rearrange("p (r c) -> p r c", c=WP)
            dma(m3[:, :, 0:1],
                zt[0:3, 0:WP].rearrange("p (r c) -> p r c", c=1))
            dma(m3[:, :, WP - 1 : WP],
                zt[0:3, 0:WP].rearrange("p (r c) -> p r c", c=1))

            def pA(k):
                return pscal[:, k : k + 1]

            # ================= E1 + E2 (elementwise) =================
            with tc.tile_pool(name="ew", bufs=1) as ew, \
                 tc.tile_pool(name="psB", bufs=1, space="PSUM") as psB:
                def pl(name):
                    return ew.tile([128, 512], F32, tag="pl_" + name, name="pl_" + name)

                ps_p = psB.tile([128, 16], F32, tag="t_bc", name="t_bc")
                nc.tensor.matmul(ps_p[:], ones_row[:], pj_src[:],
                                 start=True, stop=True)
                s.copy(pscal[:], ps_p[:])

                pj = [pl(f"J{c}") for c in range(3)]
                for c in range(3):
                    dma(pj[c][:], J3[c])
                pu, pv = pl("u"), pl("v")
                pw1, pw2 = pl("w1"), pl("w2")
                dma(pu[:], u1[:]); dma(pv[:], v1[:])
                dma(pw1[:], w11[:]); dma(pw2[:], w21[:])

                js = [pl(f"js{k}") for k in range(3)]
                for k in range(3):
                    v.tensor_scalar_mul(js[k][:], pj[0][:], pA(3 * k + 0))
                    v.scalar_tensor_tensor(js[k][:], pj[1][:], pA(3 * k + 1),
                                           js[k][:], ALU.mult, ALU.add)
                    v.scalar_tensor_tensor(js[k][:], pj[2][:], pA(3 * k + 2),
                                           js[k][:], ALU.mult, ALU.add)
                jl = js[2]

                x1, x2 = pl("x1"), pl("x2")
                v.scalar_tensor_tensor(x1[:], jl[:], -1.0, pu[:],
                                       ALU.mult, ALU.add)
                v.tensor_scalar_add(x1[:], x1[:], pA(9))
                v.scalar_tensor_tensor(x2[:], jl[:], -1.0, pv[:],
                                       ALU.mult, ALU.add)
                v.tensor_scalar_add(x2[:], x2[:], pA(9))

                tmp, tmp2 = pl("tmp"), pl("tmp2")
                # u_new / v_new (soft threshold)
                v.tensor_scalar(tmp[:], x1[:], 1.0 / lam1, -1.0 / lam1,
                                ALU.min, ALU.max)
                v.tensor_sub(tmp[:], x1[:], tmp[:])
                dma(o_u[:], tmp[:])
                v.tensor_scalar(tmp[:], x2[:], 1.0 / lam2, -1.0 / lam2,
                                ALU.min, ALU.max)
                v.tensor_sub(tmp[:], x2[:], tmp[:])
                dma(o_v[:], tmp[:])
                # w_new (clip)
                v.tensor_scalar(tmp[:], x1[:], lam1, 1.0, ALU.mult, ALU.min)
                v.tensor_scalar_max(tmp[:], tmp[:], -1.0)
                dma(o_w1[:], tmp[:])
                v.tensor_scalar(tmp[:], x2[:], lam2, 1.0, ALU.mult, ALU.min)
                v.tensor_scalar_max(tmp[:], tmp[:], -1.0)
                dma(o_w2[:], tmp[:])

                # new sorted-channel values
                jm_n, js_n = pl("jm_n"), pl("js_n")
                v.scalar_tensor_tensor(tmp[:], pw1[:], -1.0 / lam1, x1[:],
                                       ALU.mult, ALU.add)
                v.tensor_mul(tmp[:], tmp[:], jl[:])
                v.tensor_add(jm_n[:], tmp[:], js[1][:])
                v.scalar_tensor_tensor(tmp[:], pw2[:], -1.0 / lam2, x2[:],
                                       ALU.mult, ALU.add)
                v.tensor_mul(tmp[:], tmp[:], jl[:])
                v.tensor_add(js_n[:], tmp[:], js[0][:])

                comp = [js_n, jm_n, jl]
                jsc = [pl(f"jsc{c}") for c in range(3)]
                for c in range(3):
                    v.tensor_scalar_mul(jsc[c][:], comp[0][:], pA(0 + c))
                    v.scalar_tensor_tensor(jsc[c][:], comp[1][:], pA(3 + c),
                                           jsc[c][:], ALU.mult, ALU.add)
                    v.scalar_tensor_tensor(jsc[c][:], comp[2][:], pA(6 + c),
                                           jsc[c][:], ALU.mult, ALU.add)

                # ---- E2 ----
                pi = [pl(f"I{c}") for c in range(3)]
                pt = [pl(f"t{c}") for c in range(3)]
                for c in range(3):
                    dma(pi[c][:], I3[c]); dma(pt[c][:], t3[c])
                for c in range(3):
                    pbp = pl(f"bp{c}")
                    dma(pbp[:], bp3[c])
                    ta = pl(f"e2a{c}")
                    tb = pl(f"e2b{c}")
                    omt = pl(f"omt{c}")
                    v.tensor_scalar(omt[:], pt[c][:], -1.0, 1.0,
                                    ALU.mult, ALU.add)
                    v.tensor_mul(ta[:], jsc[c][:], pt[c][:])
                    v.tensor_sub(ta[:], ta[:], pi[c][:])
                    v.tensor_mul(ta[:], ta[:], omt[:])
                    v.scalar_tensor_tensor(ta[:], pbp[:], GAMMA_1, ta[:],
                                           ALU.mult, ALU.subtract)
                    v.tensor_mul(tb[:], omt[:], omt[:])
                    v.tensor_scalar_add(tb[:], tb[:], GAMMA_1)
                    rcp = pl(f"rcp{c}")
                    v.reciprocal(rcp[:], tb[:])
                    v.tensor_mul(ta[:], ta[:], rcp[:])
                    prt = pl(f"prt{c}")
                    v.tensor_reduce(prt[:, 0:1], ta[:],
                                    mybir.AxisListType.X, ALU.add)
                    ps_m = psB.tile([1, 1], F32, tag="t_m", name="t_m")
                    nc.tensor.matmul(ps_m[:], prt[:, 0:1], ones_col[:],
                                     start=True, stop=True)
                    s.mul(msrc[0:1, c : c + 1], ps_m[:], 1.0 / HW)
                ps_b = psB.tile([128, 4], F32, tag="t_bc2", name="t_bc2")
                nc.tensor.matmul(ps_b[:], ones_row[:], msrc[:],
                                 start=True, stop=True)
                s.copy(bscal[:], ps_b[:])
                dma(o_bm[:], msrc[0:1, 0:3])

                tt = [pl(f"tt{c}") for c in range(3)]
                prcs = [pl(f"Rc{c}") for c in range(3)]
                for c in range(3):
                    pzc = pl(f"Zc{c}")
                    dma(pzc[:], Z3[c]); dma(prcs[c][:], R3[c])
                    ta = pl(f"e2c{c}")
                    tb = pl(f"e2d{c}")
                    jb = pl(f"jb{c}")
                    v.tensor_scalar_sub(jb[:], jsc[c][:],
                                        bscal[:, c : c + 1])
                    v.tensor_scalar(ta[:], pi[c][:], -1.0,
                                    bscal[:, c : c + 1], ALU.mult, ALU.add)
                    v.tensor_mul(ta[:], ta[:], jb[:])
                    v.scalar_tensor_tensor(tb[:], pzc[:], eta, prcs[c][:],
                                           ALU.mult, ALU.subtract)
                    ptp = pl(f"tpc{c}")
                    dma(ptp[:], tp3[c])
                    v.scalar_tensor_tensor(tb[:], ptp[:], GAMMA_2, tb[:],
                                           ALU.mult, ALU.add)
                    v.tensor_sub(tb[:], tb[:], ta[:])
                    v.tensor_mul(ta[:], jb[:], jb[:])
                    v.tensor_scalar_add(ta[:], ta[:], GAMMA_2 + eta)
                    rcp2 = pl(f"rcp2{c}")
                    v.reciprocal(rcp2[:], ta[:])
                    v.tensor_mul(tt[c][:], tb[:], rcp2[:])

                v.tensor_scalar_mul(tmp[:], tt[0][:], wt0)
                v.scalar_tensor_tensor(tmp[:], tt[1][:], wt1, tmp[:],
                                       ALU.mult, ALU.add)
                v.scalar_tensor_tensor(tmp[:], tt[2][:], wt2, tmp[:],
                                       ALU.mult, ALU.add)
                s.activation(t1[:], tmp[:], ACT.Relu)
                dma(o_t1[:], t1[:])

                rdj = pl("rdj")
                v.tensor_mul(tmp[:], t1[:], t1[:])
                v.tensor_scalar_add(tmp[:], tmp[:], beta)
                v.reciprocal(rdj[:], tmp[:])
                omt1 = pl("omt1")
                v.tensor_scalar(omt1[:], t1[:], -1.0, 1.0, ALU.mult, ALU.add)

                for c in range(3):
                    py = pl(f"Yc{c}")
                    pq = pl(f"Qc{c}")
                    dma(py[:], Y3[c]); dma(pq[:], Q3[c])
                    ta = pl(f"e2e{c}")
                    tb = pl(f"e2f{c}")
                    v.tensor_scalar_mul(ta[:], omt1[:], bscal[:, c : c + 1])
                    v.tensor_sub(ta[:], ta[:], pi[c][:])
                    v.tensor_mul(ta[:], ta[:], t1[:])
                    v.scalar_tensor_tensor(tb[:], py[:], beta, pq[:],
                                           ALU.mult, ALU.subtract)
                    v.tensor_sub(tb[:], tb[:], ta[:])
                    jn = pl(f"jn{c}")
                    v.tensor_mul(jn[:], tb[:], rdj[:])
                    dma(o_j[c], jn[:])
                    v.tensor_sub(ta[:], jn[:], py[:])
                    v.scalar_tensor_tensor(ta[:], ta[:], beta, pq[:],
                                           ALU.mult, ALU.add)
                    dma(o_q[c], ta[:])
                    # Zin
                    v.scalar_tensor_tensor(tb[:], prcs[c][:], 1.0 / eta,
                                           t1[:], ALU.mult, ALU.add)
                    zb = ew.tile([128, 512], BF16, tag=f"pl_zinbf{c}",
                                 name=f"pl_zinbf{c}")
                    zb_b = zb[:]
                    v.tensor_copy(zb_b, tb[:])
                    dst = maps["zin"][c : c + 1, 1 : 1 + PADHW].rearrange(
                        "a (r c) -> a r c", c=WP)[0:1, 1:257, 1:257]
                    dst = dst.rearrange("a (p r) c -> (a p) r c", p=128)
                    src = zb_b.rearrange("p (r c) -> p r c", c=256)
                    dma(dst, src)

            # ================= RDN conv stack =================
            CIN_COLS = (R + 2) * WP + 12  # 8784
            with tc.tile_pool(name="cin", bufs=3) as cin, \
                 tc.tile_pool(name="cout", bufs=3) as cout, \
                 tc.tile_pool(name="cres", bufs=2) as cres, \
                 tc.tile_pool(name="psA", bufs=8, space="PSUM") as psA:

                def tiles_of_strip():
                    q0 = 0
                    out = []
                    while q0 < SLEN:
                        out.append((q0, min(512, SLEN - q0)))
                        q0 += 512
                    return out

                def store_interior(src_buf, dst_map, y0, Cn):
                    s3 = src_buf[:].rearrange("p (r c) -> p r c", c=WP)
                    v.memset(s3[:, :, 0:1], 0.0)
                    v.memset(s3[:, :, WP - 1 : WP], 0.0)
                    nc.scalar.dma_start(
                        dst_map[:, (y0 + 1) * WP : (y0 + 1 + R) * WP],
                        src_buf[:])

                def conv3x3(src_map, dst_map, wp_t, ws_t, bias_t, relu,
                            resid_map=None):
                    M = 64
                    for st in range(NSTRIP):
                        y0 = st * R
                        it = cin.tile([128, CIN_COLS], BF16, tag="cin", name="cin")
                        dma(it[0:64, 1 : 1 + (R + 2) * WP],
                            src_map[:, y0 * WP : (y0 + R + 2) * WP])
                        nc.scalar.dma_start(
                            it[64:128, 1 : 1 + (R + 1) * WP],
                            it[0:64, 1 + WP : 1 + (R + 2) * WP])
                        ob = cout.tile([64, SLEN], BF16, tag="cout", name="cout")
                        if resid_map is not None:
                            rs = cres.tile([64, SLEN], BF16, tag="cres", name="cres")
                            dma(rs[:], resid_map[:,
                                (y0 + 1) * WP : (y0 + 1) * WP + SLEN])
                        tl = tiles_of_strip()
                        for g0 in range(0, len(tl), 4):
                            grp = tl[g0 : g0 + 4]
                            pss = [psA.tile([64, n], F32, tag="ps",
                                            name="ps") for (_, n) in grp]
                            for j in range(3):
                                for (q0, n), ps in zip(grp, pss):
                                    nc.tensor.matmul(
                                        ps[:], wp_t[:, j * M : (j + 1) * M],
                                        it[0:128, q0 + j : q0 + j + n],
                                        start=(j == 0), stop=False)
                            for j in range(3):
                                for (q0, n), ps in zip(grp, pss):
                                    nc.tensor.matmul(
                                        ps[:],
                                        ws_t[0:64, j * M : (j + 1) * M],
                                        it[0:64,
                                           q0 + 516 + j : q0 + 516 + j + n],
                                        start=False, stop=(j == 2))
                            for gi, ((q0, n), ps) in enumerate(
                                    zip(grp, pss)):
                                obs = ob[:, q0 : q0 + n]
                                if gi % 2 == 0:
                                    s.activation(
                                        obs, ps[:],
                                        ACT.Relu if relu else ACT.Identity,
                                        bias=bias_t[:])
                                elif relu:
                                    v.tensor_scalar(obs, ps[:], bias_t[:],
                                                    0.0, ALU.add, ALU.max)
                                else:
                                    v.tensor_scalar_add(obs, ps[:],
                                                        bias_t[:])
                                if resid_map is not None:
                                    v.tensor_add(obs, obs,
                                                 rs[:, q0 : q0 + n])
                        store_interior(ob, dst_map, y0, 64)

                def conv1x1(a_map, b_map, dst_map, w_t, bias_t, resid):
                    for st in range(NSTRIP):
                        y0 = st * R
                        it = cin.tile([128, CIN_COLS], BF16, tag="cin", name="cin")
                        dma(it[0:64, 0:SLEN],
                            a_map[:, (y0 + 1) * WP : (y0 + 1) * WP + SLEN])
                        nc.scalar.dma_start(
                            it[64:128, 0:SLEN],
                            b_map[:, (y0 + 1) * WP : (y0 + 1) * WP + SLEN])
                        ob = cout.tile([64, SLEN], BF16, tag="cout", name="cout")
                        for ti, (q0, n) in enumerate(tiles_of_strip()):
                            ps = psA.tile([64, n], F32, tag="ps", name="ps")
                            nc.tensor.matmul(ps[:], w_t[:],
                                             it[0:128, q0 : q0 + n],
                                             start=True, stop=True)
                            obs = ob[:, q0 : q0 + n]
                            if ti % 2 == 0:
                                s.activation(obs, ps[:], ACT.Identity,
                                             bias=bias_t[:])
                            else:
                                v.tensor_scalar_add(obs, ps[:], bias_t[:])
                            if resid:
                                v.tensor_add(obs, obs, it[0:64, q0 : q0 + n])
                        store_interior(ob, dst_map, y0, 64)

                def conv_sfe1(src_map, dst_map, w_t, bias_t):
                    D = 2
                    for st in range(NSTRIP):
                        y0 = st * R
                        it = cin.tile([27, CIN_COLS], BF16, tag="cin9", name="cin9")
                        for a in range(3):
                            src3 = bass.AP(
                                src_map.tensor, y0 * WP + a * WP,
                                [[1, 3], [PADHW + 2, 3], [1, SLEN]])
                            dma(it[9 * a : 9 * a + 9, D : D + SLEN], src3)
                        ob = cout.tile([64, SLEN], BF16, tag="cout", name="cout")
                        for (q0, n) in tiles_of_strip():
                            ps = psA.tile([64, n], F32, tag="ps", name="ps")
                            nc.tensor.matmul(ps[:], w_t[:],
                                             it[0:27, D + q0 : D + q0 + n],
                                             start=True, stop=True)
                            s.activation(ob[:, q0 : q0 + n], ps[:],
                                         ACT.Identity, bias=bias_t[:])
                        store_interior(ob, dst_map, y0, 64)

                def conv_out(src_map, wp_t, ws_t, bias_t):
                    for st in range(NSTRIP):
                        y0 = st * R
                        it = cin.tile([128, CIN_COLS], BF16, tag="cin", name="cin")
                        dma(it[0:64, 1 : 1 + (R + 2) * WP],
                            src_map[:, y0 * WP : (y0 + R + 2) * WP])
                        nc.scalar.dma_start(
                            it[64:128, 1 : 1 + (R + 1) * WP],
                            it[0:64, 1 + WP : 1 + (R + 2) * WP])
                        zb = cres.tile([3, SLEN], BF16, tag="zbuf", name="zbuf", bufs=1)
                        tl = tiles_of_strip()
                        for g0 in range(0, len(tl), 4):
                            grp = tl[g0 : g0 + 4]
                            pss = [psA.tile([3, n], F32, tag="ps",
                                            name="ps") for (_, n) in grp]
                            for j in range(3):
                                for (q0, n), ps in zip(grp, pss):
                                    nc.tensor.matmul(
                                        ps[:], wp_t[:, j * 3 : (j + 1) * 3],
                                        it[0:128, q0 + j : q0 + j + n],
                                        start=(j == 0), stop=False)
                            for j in range(3):
                                for (q0, n), ps in zip(grp, pss):
                                    nc.tensor.matmul(
                                        ps[:],
                                        ws_t[0:64, j * 3 : (j + 1) * 3],
                                        it[0:64,
                                           q0 + 516 + j : q0 + 516 + j + n],
                                        start=False, stop=(j == 2))
                            for (q0, n), ps in zip(grp, pss):
                                s.activation(zb[:, q0 : q0 + n], ps[:],
                                             ACT.Identity, bias=bias_t[:])
                        src = zb[:].rearrange("p (r c) -> p r c", c=WP)
                        src = src[:, :, 1:257]
                        dst = zmap[:].rearrange("p (r c) -> p r c", c=256)
                        dst = dst[:, y0 : y0 + R, :]
                        dma(dst, src)

                conv_sfe1(maps["zin"], maps["sfe1"], wt["sfe1"], bt["sfe1"])
                conv3x3(maps["sfe1"], maps["h0"], wt["sfe2_p"], wt["sfe2_s"],
                        bt["sfe2"], relu=False)
                conv3x3(maps["h0"], maps["d0"], wt["d0_p"], wt["d0_s"],
                        bt["d0"], relu=True)
                conv1x1(maps["h0"], maps["d0"], maps["h1"], wt["l0"],
                        bt["l0"], resid=True)
                conv3x3(maps["h1"], maps["d1"], wt["d1_p"], wt["d1_s"],
                        bt["d1"], relu=True)
                conv1x1(maps["h1"], maps["d1"], maps["h2"], wt["l1"],
                        bt["l1"], resid=True)
                conv1x1(maps["h1"], maps["h2"], maps["g"], wt["g1"],
                        bt["g1"], resid=False)
                conv3x3(maps["g"], maps["g2"], wt["g2_p"], wt["g2_s"],
                        bt["g2"], relu=False, resid_map=maps["sfe1"])
                conv_out(maps["g2"], wt["out_p"], wt["out_s"], bt["out"])

            # ================= E3 =================
            with tc.tile_pool(name="e3", bufs=2) as e3:
                for c in range(3):
                    rp = e3.tile([128, 512], F32, tag="rp", name="rp")
                    dma(rp[:], R3[c])
                    zpb = e3.tile([128, 512], BF16, tag="zpb", name="zpb")
                    dma(zpb[:], zmap[:].rearrange(
                        "a (p c) -> a p c", c=512)[c])
                    zp = e3.tile([128, 512], F32, tag="zp", name="zp")
                    v.tensor_copy(zp[:], zpb[:])
                    dma(o_z[c], zp[:])
                    rn = e3.tile([128, 512], F32, tag="rn", name="rn")
                    v.tensor_sub(rn[:], t1[:], zp[:])
                    v.scalar_tensor_tensor(rn[:], rn[:], eta, rp[:],
                                           ALU.mult, ALU.add)
                    dma(o_r[c], rn[:])

    nc.finalize()
    _split_sync_waits(nc, max_waits=1)
    return nc


_CACHE = {}


def _pack_weights(params):
    def bf(x):
        return np.ascontiguousarray(x.astype(_BF))

    out = {}
    for nm, key in (("sfe2", "sfe2_w"), ("d0", "rdb0_dense_w"),
                    ("d1", "rdb1_dense_w"), ("g2", "gff2_w")):
        Wt = np.asarray(params[key], np.float32)  # [64,64,3,3]
        wp = np.zeros((128, 192), np.float32)
        ws = np.zeros((64, 192), np.float32)
        for j in range(3):
            wp[0:64, j * 64 : (j + 1) * 64] = Wt[:, :, 0, j].T
            wp[64:128, j * 64 : (j + 1) * 64] = Wt[:, :, 1, j].T
            ws[0:64, j * 64 : (j + 1) * 64] = Wt[:, :, 2, j].T
        out["w_" + nm + "_p"] = bf(wp)
        out["w_" + nm + "_s"] = bf(ws)
    Wt = np.asarray(params["out_w"], np.float32)  # [3,64,3,3]
    wp = np.zeros((128, 9), np.float32)
    ws = np.zeros((64, 9), np.float32)
    for j in range(3):
        wp[0:64, j * 3 : (j + 1) * 3] = Wt[:, :, 0, j].T
        wp[64:128, j * 3 : (j + 1) * 3] = Wt[:, :, 1, j].T
        ws[0:64, j * 3 : (j + 1) * 3] = Wt[:, :, 2, j].T
    out["w_out_p"] = bf(wp)
    out["w_out_s"] = bf(ws)
    Wt = np.asarray(params["sfe1_w"], np.float32)  # [64,3,3,3]
    w9 = np.zeros((27, 64), np.float32)
    for a in range(3):
        for b in range(3):
            tk = a * 3 + b
            w9[3 * tk : 3 * tk + 3, :] = Wt[:, :, a, b].T
    out["w_sfe1"] = bf(w9)
    for nm, key in (("l0", "rdb0_lff_w"), ("l1", "rdb1_lff_w"),
                    ("g1", "gff1_w")):
        Wt = np.asarray(params[key], np.float32)  # [64,128,1,1]
        out["w_" + nm] = bf(Wt[:, :, 0, 0].T)
    for nm, key in (("sfe1", "sfe1_b"), ("sfe2", "sfe2_b"),
                    ("d0", "rdb0_dense_b"), ("l0", "rdb0_lff_b"),
                    ("d1", "rdb1_dense_b"), ("l1", "rdb1_lff_b"),
                    ("g1", "gff1_b"), ("g2", "gff2_b"), ("out", "out_b")):
        out["b_" + nm] = np.ascontiguousarray(
            np.asarray(params[key], np.float32))
    return out


def kernel(**inputs):
    params = inputs["params"]
    beta = float(np.asarray(params["beta"])[0])
    eta = float(np.asarray(params["eta"])[0])
    lam1 = float(np.asarray(params["lambda_1"])[0])
    lam2 = float(np.asarray(params["lambda_2"])[0])
    wtd = np.asarray(params["t1d_w"], np.float32)[0, :, 0, 0]
    sc = dict(beta=beta, eta=eta, lam1=lam1, lam2=lam2,
              wt0=float(wtd[0]), wt1=float(wtd[1]), wt2=float(wtd[2]))
    key = tuple(sorted(sc.items()))
    if key not in _CACHE:
        _CACHE[key] = _build_program(sc)
    nc = _CACHE[key]

    wpk = _pack_weights(params)

    f32 = lambda x: np.ascontiguousarray(np.asarray(x, np.float32))
    J = f32(inputs["J"])
    means = J.mean(axis=(2, 3), dtype=np.float32)  # [8,3]
    idx = np.argsort(means, axis=1, kind="stable")

    in_maps = []
    for b in range(NCORES):
        P = np.zeros((3, 3), np.float32)
        for k in range(3):
            P[k, idx[b, k]] = 1.0
        psc = np.zeros(16, np.float32)
        psc[0:9] = P.reshape(-1)
        psc[9] = means[b, idx[b, 2]]
        m = {
            "I3": f32(inputs["I"][b]).reshape(3, 128, 512),
            "tp3": f32(inputs["t_p"][b]).reshape(3, 128, 512),
            "bp3": f32(inputs["B_p"][b]).reshape(3, 128, 512),
            "t3": f32(inputs["t"][b]).reshape(3, 128, 512),
            "J3": J[b].reshape(3, 128, 512),
            "Y3": f32(inputs["Y"][b]).reshape(3, 128, 512),
            "Z3": f32(inputs["Z"][b]).reshape(3, 128, 512),
            "Q3": f32(inputs["Q"][b]).reshape(3, 128, 512),
            "R3": f32(inputs["R"][b]).reshape(3, 128, 512),
            "u1": f32(inputs["u"][b]).reshape(128, 512),
            "v1": f32(inputs["v"][b]).reshape(128, 512),
            "w11": f32(inputs["w_1"][b]).reshape(128, 512),
            "w21": f32(inputs["w_2"][b]).reshape(128, 512),
            "psc": psc,
        }
        m.update(wpk)
        in_maps.append(m)

    trace = bool(int(os.environ.get("KERNEL_TRACE", "0")))
    tkw = {}
    if trace:
        try:
            sys.path.insert(0, "/root/problem/work")
            import profhook

            profhook.install()
            tkw = dict(trace=True, tmpdir=os.environ.get(
                "KERNEL_TRACE_DIR", "/root/problem/work/trace_out"))
        except Exception:
            tkw = {}
    r = run_bass_kernel_spmd(nc, in_maps, list(range(NCORES)), **tkw)
    res = r.results
    if trace and getattr(r, "exec_time_ns", None) is not None:
        kernel.last_exec_time_ns = r.exec_time_ns

    B = np.empty((8, 3, H, W), np.float32)
    t_new = np.empty((8, 3, H, W), np.float32)
    Jn = np.empty((8, 3, H, W), np.float32)
    Zn = np.empty((8, 3, H, W), np.float32)
    Qn = np.empty((8, 3, H, W), np.float32)
    Rn = np.empty((8, 3, H, W), np.float32)
    un = np.empty((8, 1, H, W), np.float32)
    vn = np.empty((8, 1, H, W), np.float32)
    w1n = np.empty((8, 1, H, W), np.float32)
    w2n = np.empty((8, 1, H, W), np.float32)
    for b in range(NCORES):
        o = res[b]
        B[b] = o["o_bm"].reshape(3, 1, 1)
        t_new[b] = o["o_t1"].reshape(1, H, W)
        Jn[b] = o["o_j"].reshape(3, H, W)
        Zn[b] = o["o_z"].reshape(3, H, W)
        Qn[b] = o["o_q"].reshape(3, H, W)
        Rn[b] = o["o_r"].reshape(3, H, W)
        un[b, 0] = o["o_u"].reshape(H, W)
        vn[b, 0] = o["o_v"].reshape(H, W)
        w1n[b, 0] = o["o_w1"].reshape(H, W)
        w2n[b, 0] = o["o_w2"].reshape(H, W)
    Y = f32(inputs["Y"])
    return (B, t_new, Jn, Y, Zn, Qn, Rn, un, vn, w1n, w2n,
            f32(params["beta"]))
